# revision 44
# baseline (speedup 1.0000x reference)
"""Trainium2 Bass kernel for nn_GPTrack2D (dense transformer with linear
attention and a per-frame recurrence over L).

Sharding: batch (2) -> two groups of 4 cores; tokens (1024 -> 256/core)
within each group. Linear attention's k^T v state is all-reduced per frame
within the group; the all-reduce hides behind the previous frame's MLP and
the next frame's x-side LayerNorm (software-pipelined emission).

v2 restructure vs baseline:
- All weights except w2 are SBUF-resident per scan segment (no per-frame
  g1 streaming); w2 streams in per-mj chunks with prefetch.
- LayerNorm: stats from bf16 copies via ones-matmuls into one packed PSUM
  bank; rsqrt via Ln+Exp on ScalarE (no slow DVE reciprocal); the mean
  term is folded into each GEMM as a rank-1 (K=3) matmul with host-packed
  [bias; -colsum(W); -colsum(Wh)] rows, so normalize is a single bf16
  multiply z' = bf16(x) * rsqrt(var).
- MLP mj-loop fused (y1 chunk -> gelu -> y2 accumulate), no big y1 buffer.
- heff (h + pos) is maintained directly by the attention tail, removing
  the per-frame pos re-add from the critical chain.
- Wout pre-scaled by 256 on host so the f16 kv-state scaling (1/256)
  cancels without an extra vector op.
- Emission order per frame: h-chain, qkv+collective launch, next frame's
  x-side, previous frame's MLP, attention tail, output LN.

Precision: residual stream / carry f32; matmul operands f16 except LN
stat inputs (bf16 for range: |h| reaches ~1.3e6). kv state scaled by
1/256 to fit f16 (max |m|/sd <= 0.13 so the bf16 mean-fold is safe).
"""

import functools

import numpy as np

import concourse.bacc as bacc
import concourse.mybir as mybir
from concourse import tile
from concourse.bass_utils import run_bass_kernel_spmd

F32 = mybir.dt.float32
BF16 = mybir.dt.bfloat16
F16 = mybir.dt.float16
AF = mybir.ActivationFunctionType
ALU = mybir.AluOpType

B, L, N, D, M, H = 2, 12, 1024, 768, 3072, 12
NCORES = 8
GROUP = 4                 # cores per batch group
TOK = N // GROUP          # 256 tokens per core
KT = D // 128             # 6 d-tiles
MT = M // 128             # 24 m-tiles
F3 = 3 * D                # 2304
EPS = 1e-5
KVS = 1.0 / 256.0         # kv-state scale so fp16 holds it
KVSI = 256.0

# dev-scale knobs (full problem: L_RUN=12, LAYERS_RUN=2, DIRS_RUN=(0, 1))
L_RUN = L
LAYERS_RUN = 2
DIRS_RUN = (0, 1)
RES_MJ = 12               # g1 m-tiles resident in SBUF; the rest stream

REPLICA_GROUPS = [[0, 1, 2, 3], [4, 5, 6, 7]]


# ---------------------------------------------------------------- host prep

def _pack_weights(inputs, dtype=np.float16):
    segs = []
    for layer in range(LAYERS_RUN):
        for d in DIRS_RUN:
            gi = np.asarray(inputs["lni_g"][d, layer]); bi = np.asarray(inputs["lni_b"][d, layer])
            gh = np.asarray(inputs["lnh_g"][d, layer]); bh = np.asarray(inputs["lnh_b"][d, layer])
            go = np.asarray(inputs["lno_g"][d, layer]); bo = np.asarray(inputs["lno_b"][d, layer])
            Wqkv = np.asarray(inputs["Wqkv"][d, layer]); bqkv = np.asarray(inputs["bqkv"][d, layer])
            Wqkvh = np.asarray(inputs["Wqkvh"][d, layer]); bqkvh = np.asarray(inputs["bqkvh"][d, layer])
            Wout = np.asarray(inputs["Wout"][d, layer]); bout = np.asarray(inputs["bout"][d, layer])
            W1 = np.asarray(inputs["W1"][d, layer]); b1 = np.asarray(inputs["b1"][d, layer])
            W2 = np.asarray(inputs["W2"][d, layer]); b2 = np.asarray(inputs["b2"][d, layer])

            gqkv = gi[:, None] * Wqkv                      # (D, 3D)
            gqkvh = gh[:, None] * Wqkvh
            cqkv = bi @ Wqkv + bqkv + bh @ Wqkvh + bqkvh   # (3D,)
            # rows live on partitions 0/32/64 (DVE base-partition rule);
            # the zero rows annihilate whatever sits in the R3 tile between
            B3 = np.zeros((65, F3), np.float32)
            B3[0], B3[32], B3[64] = cqkv, -gqkv.sum(0), -gqkvh.sum(0)
            g1 = go[:, None] * W1                          # (D, M)
            c1 = bo @ W1 + b1                              # (M,)
            B2 = np.zeros((33, M), np.float32)
            B2[0], B2[32] = c1, -g1.sum(0)

            seg = dict(
                # (128, KT, F3): [p, kd, f] = gqkv[kd*128+p, f]
                gqkv=np.ascontiguousarray(
                    gqkv.reshape(KT, 128, F3).transpose(1, 0, 2)).astype(dtype),
                gqkvh=np.ascontiguousarray(
                    gqkvh.reshape(KT, 128, F3).transpose(1, 0, 2)).astype(dtype),
                B3=B3.astype(dtype),
                # wout pre-scaled by KVSI: cancels the f16 kv-state 1/256.
                # ft-chunked for streaming: [ft, p, kd, c] = w[kd*128+p, ft*128+c]
                wout=np.ascontiguousarray(
                    (Wout * KVSI).reshape(KT, 128, KT, 128)
                    .transpose(2, 1, 0, 3)).astype(dtype),
                bout=np.ascontiguousarray(
                    bout.reshape(KT, 128).T).astype(np.float32),
                # resident half: (128, RES_MJ, KT, 128)
                g1=np.ascontiguousarray(
                    g1.reshape(KT, 128, MT, 128)
                    .transpose(1, 2, 0, 3)[:, :RES_MJ]).astype(dtype),
                # streamed half: (MT-RES_MJ, 128, KT, 128)
                g1s=np.ascontiguousarray(
                    g1.reshape(KT, 128, MT, 128)
                    .transpose(2, 1, 0, 3)[RES_MJ:]).astype(dtype),
                B2=B2.astype(dtype),
                w2=W2.reshape(MT, 128, D).astype(dtype),          # (MT, 128, D)
                b2=np.ascontiguousarray(
                    b2.reshape(KT, 128).T).astype(np.float32),    # (128, KT)
            )
            segs.append(seg)
    return segs


def _feat_major(a, dtype):
    """(..., tok, D) -> (..., 128, KT, tok) tiled feature-major."""
    t = np.moveaxis(np.asarray(a), -1, -2)                # (..., D, tok)
    shp = t.shape[:-2]
    t = t.reshape(shp + (KT, 128, t.shape[-1]))           # (..., KT, 128, tok)
    t = np.moveaxis(t, -3, -2)                            # (..., 128, KT, tok)
    return np.ascontiguousarray(t).astype(dtype)


def make_in_maps(inputs):
    segs = _pack_weights(inputs)
    in_maps = []
    for core in range(NCORES):
        b = core // GROUP
        s = (core % GROUP) * TOK
        m = {}
        m["x_in"] = _feat_major(
            np.asarray(inputs["x"])[b, :L_RUN, s:s + TOK, :], np.float32)
        m["h0_in"] = _feat_major(
            np.asarray(inputs["hidden"])[b, s:s + TOK, :], np.float32)
        m["spat"] = _feat_major(
            np.asarray(inputs["spatial_pos"])[b, s:s + TOK, :], np.float32)
        tp = np.asarray(inputs["temporal_pos"])[b, :L_RUN, :]   # (L, D)
        tp = tp.T.reshape(KT, 128, L_RUN).transpose(1, 0, 2)
        m["tpos"] = np.ascontiguousarray(tp).astype(np.float32)  # (128, KT, L)
        for si, seg in enumerate(segs):
            for k, v in seg.items():
                m[f"{k}_{si}"] = v
        in_maps.append(m)
    return in_maps


def unshard_output(results):
    out = np.empty((B, L_RUN, N, D), np.float32)
    for core in range(NCORES):
        b = core // GROUP
        s = (core % GROUP) * TOK
        o = np.asarray(results[core]["out_x"])            # (L, 128, KT, TOK)
        o = o.transpose(0, 2, 1, 3).reshape(L_RUN, D, TOK)
        out[b, :, s:s + TOK, :] = np.moveaxis(o, -1, -2)
    return out


# ---------------------------------------------------------------- kernel build

class Ctx:
    """Pools, constants and persistent tiles used during emission."""


def _ln_chain(nc, cx, src32, tag):
    """Feature-major LN for an SBUF (128, KT, TOK) f32 tile.

    Emits: bf16 copy xb, squares, packed stats matmuls (s1|s2 in one
    PSUM bank), mean/var smalls, rsqrt via Ln+Exp. Returns (xb, rbb,
    mrow, rb32): xb bf16 copy, rbb bf16 (128,TOK) rsqrt, plus the f32
    mean/rb tiles for the mrb row write.
    """
    # xq packs [bf16 copy | its square] so one 512-wide matmul per kd
    # yields both stat sums; squares on DVE keep ScalarE's LUT unthrashed
    xq = cx.act1.tile([128, KT, 2 * TOK], BF16, name=f"xb_{tag}", tag=f"xb_{tag}")
    s12 = cx.psA.tile([128, 2 * TOK], F32, name="ps", tag="ps")
    for kd in range(KT):
        nc.vector.tensor_copy(xq[:, kd, 0:TOK], src32[:, kd, :])
        nc.vector.tensor_mul(xq[:, kd, TOK:2 * TOK], xq[:, kd, 0:TOK],
                             xq[:, kd, 0:TOK])
        nc.tensor.matmul(s12[:], cx.onesB[:], xq[:, kd, :],
                         start=(kd == 0), stop=(kd == KT - 1))
    mean = cx.sm.tile([128, TOK], F32, name="mean", tag="lnsm")
    nc.vector.tensor_scalar_mul(mean[:], s12[:, 0:TOK], 1.0 / D)
    msq = cx.sm.tile([128, TOK], F32, name="msq", tag="lnsm")
    nc.vector.tensor_mul(msq[:], mean[:], mean[:])
    ve = cx.sm.tile([128, TOK], F32, name="ve", tag="lnsm")
    nc.vector.scalar_tensor_tensor(ve[:], s12[:, TOK:2 * TOK], 1.0 / D, msq[:],
                                   op0=ALU.mult, op1=ALU.subtract)
    sd = cx.sm.tile([128, TOK], F32, name="sd", tag="lnsm")
    nc.scalar.activation(sd[:], ve[:], AF.Sqrt, bias=cx.epsc[:])
    rb32 = cx.sm.tile([128, TOK], F32, name="rb32", tag="lnsm")
    nc.vector.reciprocal(rb32[:], sd[:])
    rbb = cx.tmp.tile([128, TOK], BF16, name=f"rbb_{tag}", tag=f"rbb_{tag}")
    nc.vector.tensor_copy(rbb[:], rb32[:])
    return xq, rbb, mean, rb32


def _normalize(nc, cx, pool, xq, rbb, tag, bufs_tag=None):
    """z'[kd] = xb[kd] * rbb -> f16 (128, KT, TOK)."""
    z = pool.tile([128, KT, TOK], F16, name=f"z_{tag}",
                  tag=bufs_tag or f"z_{tag}")
    for kd in range(KT):
        nc.vector.tensor_mul(z[:, kd, :], xq[:, kd, 0:TOK], rbb[:])
    return z


def _elu1(nc, cx, psum_ap, out_ap, ncols):
    """out = elu(psum)+1 = exp(min(x,0)) + max(x,0)."""
    tmin = cx.act1.tile([128, 512], F32, name="emin", tag="emin")
    texp = cx.act1.tile([128, 512], F32, name="eexp", tag="eexp")
    nc.vector.tensor_scalar_min(tmin[:, :ncols], psum_ap, 0.0)
    nc.scalar.activation(texp[:, :ncols], tmin[:, :ncols], AF.Exp)
    nc.vector.scalar_tensor_tensor(out_ap, psum_ap, 0.0, texp[:, :ncols],
                                   op0=ALU.max, op1=ALU.add)


def build_nc():
    nc = bacc.Bacc("TRN2", target_bir_lowering=False, debug=False,
                   num_devices=NCORES)

    x_in = nc.dram_tensor("x_in", [L_RUN, 128, KT, TOK], F32, kind="ExternalInput")
    h0_in = nc.dram_tensor("h0_in", [128, KT, TOK], F32, kind="ExternalInput")
    spat = nc.dram_tensor("spat", [128, KT, TOK], F32, kind="ExternalInput")
    tpos = nc.dram_tensor("tpos", [128, KT, L_RUN], F32, kind="ExternalInput")
    nseg = LAYERS_RUN * len(DIRS_RUN)
    segs = []
    for si in range(nseg):
        segs.append(dict(
            gqkv=nc.dram_tensor(f"gqkv_{si}", [128, KT, F3], F16, kind="ExternalInput"),
            gqkvh=nc.dram_tensor(f"gqkvh_{si}", [128, KT, F3], F16, kind="ExternalInput"),
            B3=nc.dram_tensor(f"B3_{si}", [65, F3], F16, kind="ExternalInput"),
            wout=nc.dram_tensor(f"wout_{si}", [KT, 128, KT, 128], F16, kind="ExternalInput"),
            bout=nc.dram_tensor(f"bout_{si}", [128, KT], F32, kind="ExternalInput"),
            g1=nc.dram_tensor(f"g1_{si}", [128, RES_MJ, KT, 128], F16, kind="ExternalInput"),
            g1s=nc.dram_tensor(f"g1s_{si}", [MT - RES_MJ, 128, KT, 128], F16, kind="ExternalInput"),
            B2=nc.dram_tensor(f"B2_{si}", [33, M], F16, kind="ExternalInput"),
            w2=nc.dram_tensor(f"w2_{si}", [MT, 128, D], F16, kind="ExternalInput"),
            b2=nc.dram_tensor(f"b2_{si}", [128, KT], F32, kind="ExternalInput"),
        ))
    out_x = nc.dram_tensor("out_x", [L_RUN, 128, KT, TOK], F32, kind="ExternalOutput")

    with tile.TileContext(nc) as tc:
        with (
            tc.tile_pool(name="cst", bufs=1) as cst,
            tc.tile_pool(name="wt", bufs=1) as wt,
            tc.tile_pool(name="stream", bufs=3) as stream,
            tc.tile_pool(name="act1", bufs=1) as act1,
            tc.tile_pool(name="act2", bufs=2) as act2,
            tc.tile_pool(name="state", bufs=1) as state,
            tc.tile_pool(name="tmp", bufs=2) as tmp,
            tc.tile_pool(name="sm", bufs=6) as sm,
            tc.tile_pool(name="psA", bufs=4, space="PSUM") as psA,
            tc.tile_pool(name="psY", bufs=3, space="PSUM") as psY,
            tc.tile_pool(name="dram", bufs=2, space="DRAM") as dram,
        ):
            cx = Ctx()
            cx.wt, cx.stream, cx.act1, cx.act2 = wt, stream, act1, act2
            cx.state, cx.tmp, cx.sm = state, tmp, sm
            cx.psA, cx.psY, cx.dram = psA, psY, dram

            cx.onesB = cst.tile([128, 128], BF16, name="onesB")
            nc.vector.memset(cx.onesB[:], 1.0)
            cx.epsc = cst.tile([128, 1], F32, name="epsc")
            nc.vector.memset(cx.epsc[:], EPS)
            cx.spat = cst.tile([128, KT, TOK], F32, name="spatc")
            nc.sync.dma_start(cx.spat[:], spat.ap())
            cx.tpos = cst.tile([128, KT, L_RUN], F32, name="tposc")
            nc.sync.dma_start(cx.tpos[:], tpos.ap())
            # block-diag kv holder: off-diagonal blocks stay zero forever
            cx.bd16 = state.tile([128, KT, 128], F16, name="bd16", tag="bd16")
            nc.vector.memset(cx.bd16[:], 0.0)
            cx.h0_in = h0_in

            x1_sc = dram.tile([L_RUN, 128, KT, TOK], F32, name="x1_sc", tag="x1_sc")
            yf_sc = dram.tile([L_RUN, 128, KT, TOK], F32, name="yf_sc", tag="yf_sc")

            for layer in range(LAYERS_RUN):
                x_src = x_in.ap() if layer == 0 else x1_sc
                last_layer = layer == LAYERS_RUN - 1
                for dir_i, d in enumerate(DIRS_RUN):
                    si = layer * len(DIRS_RUN) + dir_i
                    fwd = d == 0
                    last_scan = dir_i == len(DIRS_RUN) - 1
                    frames = (list(range(L_RUN)) if fwd
                              else list(range(L_RUN - 1, -1, -1)))
                    if not last_scan:
                        out_dst = yf_sc
                    elif last_layer:
                        out_dst = out_x.ap()
                    else:
                        out_dst = x1_sc
                    _emit_scan(nc, cx, segs[si], x_src, frames,
                               pos_fixed=(layer if fwd else None),
                               yf_sc=yf_sc, fwd=fwd, out_dst=out_dst)
    nc.compile()
    return nc


def _emit_scan(nc, cx, seg, x_src, frames, pos_fixed, yf_sc, fwd,
               out_dst):
    w = {}
    for nm, shape, dt in (("gqkv", [128, KT, F3], F16),
                          ("gqkvh", [128, KT, F3], F16),
                          ("B3", [65, F3], F16),
                          ("g1", [128, RES_MJ, KT, 128], F16),
                          ("B2", [33, M], F16),
                          ("bout", [128, KT], F32),
                          ("b2", [128, KT], F32)):
        w[nm] = cx.wt.tile(shape, dt, name=nm, tag=nm)
        nc.sync.dma_start(w[nm][:], seg[nm].ap())

    # heff = h0 + pos[tp0] (f32 carry, maintained by the attention tail);
    # h0 borrows the yfld slot (idle at scan starts)
    h0t = cx.act1.tile([128, KT, TOK], F32, name="yfld", tag="yfld")
    nc.sync.dma_start(h0t[:], cx.h0_in.ap())
    heff = cx.state.tile([128, KT, TOK], F32, name="heff", tag="heff")
    tp0 = pos_fixed if pos_fixed is not None else frames[0]
    for kd in range(KT):
        nc.vector.scalar_tensor_tensor(
            heff[:, kd, :], cx.spat[:, kd, :], cx.tpos[:, kd, tp0:tp0 + 1],
            h0t[:, kd, :], op0=ALU.mult, op1=ALU.add)

    xs = _x_stage(nc, cx, x_src, frames[0])
    pend = None
    for i, t in enumerate(frames):
        nxt = frames[i + 1] if i + 1 < len(frames) else None
        pend, xs = _emit_frame(nc, cx, seg, w, t, nxt, x_src, heff, xs,
                               pos_fixed, yf_sc, fwd, out_dst, pend)
    _emit_mlp(nc, cx, seg, w, pend)


def _x_stage(nc, cx, x_src, t):
    """x-side work for frame t: load, add pos, LN stats, normalize.

    Returns dict(xeff, zx, R3) where R3 rows are [ones; mrb_x; <mrb_h>]
    (row 2 filled later by the h-stage).
    """
    # xeff doubles as x2 later (attn tail adds in place); lives until the
    # deferred MLP tail of this frame -> 3 buffers (stream pool)
    xeff = cx.stream.tile([128, KT, TOK], F32, name="xe", tag="xe")
    nc.sync.dma_start(xeff[:], x_src[t])
    for kd in range(KT):
        nc.vector.scalar_tensor_tensor(
            xeff[:, kd, :], cx.spat[:, kd, :], cx.tpos[:, kd, t:t + 1],
            xeff[:, kd, :], op0=ALU.mult, op1=ALU.add)
    xb, rbb, mean, rb32 = _ln_chain(nc, cx, xeff, "x")
    R3 = cx.act2.tile([65, TOK], F16, name="R3", tag="R3")
    nc.vector.memset(R3[:], 0.0)     # garbage rows x zero weights else NaN
    nc.vector.memset(R3[0:1, :], 1.0)
    # stats are partition-replicated; read partition 32 to write row 32
    nc.vector.tensor_mul(R3[32:33, :], mean[32:33, :], rb32[32:33, :])
    zx = _normalize(nc, cx, cx.act2, xb, rbb, "x")
    return dict(xeff=xeff, zx=zx, R3=R3)


def _emit_frame(nc, cx, seg, w, t, nxt, x_src, heff, xs, pos_fixed, yf_sc,
                fwd, out_dst, pend):
    tpn = pos_fixed if pos_fixed is not None else nxt   # next frame's pos idx

    xeff, zx, R3 = xs["xeff"], xs["zx"], xs["R3"]

    # ---- h-side LN (critical chain)
    hb, rbh, meanh, rb32h = _ln_chain(nc, cx, heff, "h")
    nc.vector.tensor_mul(R3[64:65, :], meanh[64:65, :], rb32h[64:65, :])
    zh = _normalize(nc, cx, cx.act1, hb, rbh, "h")

    # ---- q (feature-major)
    q16 = cx.act1.tile([128, KT, TOK], F16, name="q16", tag="q16")
    for ft in range(KT):
        ps = cx.psA.tile([128, 2 * TOK], F32, name="ps", tag="ps")
        for kd in range(KT):
            nc.tensor.matmul(ps[:, 0:TOK], w["gqkv"][:, kd, ft * 128:(ft + 1) * 128],
                             zx[:, kd, :], start=(kd == 0), stop=False)
        for kd in range(KT):
            nc.tensor.matmul(ps[:, 0:TOK], w["gqkvh"][:, kd, ft * 128:(ft + 1) * 128],
                             zh[:, kd, :], start=False, stop=False)
        nc.tensor.matmul(ps[:, 0:TOK], w["B3"][:, ft * 128:(ft + 1) * 128],
                         R3[:], start=False, stop=True)
        _elu1(nc, cx, ps[:, 0:TOK], q16[:, ft, :], TOK)

    # ---- k, v (token-major): (128, 2, D) each [tok-half, feature]
    k16 = cx.act1.tile([128, 2, D], F16, name="k16", tag="k16")
    v16 = cx.act1.tile([128, 2, D], F16, name="v16", tag="v16")
    for tok2 in range(2):
        for fc in range(3):  # chunks of 512 covering [D, 3D): k then v
            lo = D + fc * 512
            ps = cx.psA.tile([128, 2 * TOK], F32, name="ps", tag="ps")
            for kd in range(KT):
                nc.tensor.matmul(ps[:], zx[:, kd, tok2 * 128:(tok2 + 1) * 128],
                                 w["gqkv"][:, kd, lo:lo + 512],
                                 start=(kd == 0), stop=False)
            for kd in range(KT):
                nc.tensor.matmul(ps[:], zh[:, kd, tok2 * 128:(tok2 + 1) * 128],
                                 w["gqkvh"][:, kd, lo:lo + 512],
                                 start=False, stop=False)
            nc.tensor.matmul(ps[:], R3[:, tok2 * 128:(tok2 + 1) * 128],
                             w["B3"][:, lo:lo + 512], start=False, stop=True)
            off = fc * 512
            if fc == 0:
                _elu1(nc, cx, ps[:], k16[:, tok2, 0:512], 512)
            elif fc == 1:
                _elu1(nc, cx, ps[:, 0:256], k16[:, tok2, 512:768], 256)
                nc.vector.tensor_copy(v16[:, tok2, 0:256], ps[:, 256:512])
            else:
                nc.vector.tensor_copy(v16[:, tok2, 256:768], ps[:])

    # ---- kv state per head-pair; pack diag blocks into (128, 384) f32
    kvpack = cx.act1.tile([128, H * 32], F32, name="kvpack", tag="kvpack")
    for hp in range(KT):
        ps = cx.psA.tile([128, 2 * TOK], F32, name="ps", tag="ps")
        pskv = ps[:, 0:128]
        for tok2 in range(2):
            nc.tensor.matmul(pskv, k16[:, tok2, hp * 128:(hp + 1) * 128],
                             v16[:, tok2, hp * 128:(hp + 1) * 128],
                             start=(tok2 == 0), stop=(tok2 == 1))
        nc.vector.tensor_copy(kvpack[0:64, hp * 64:(hp + 1) * 64],
                              pskv[0:64, 0:64])
        nc.vector.tensor_copy(kvpack[64:128, hp * 64:(hp + 1) * 64],
                              pskv[64:128, 64:128])

    # ---- all-reduce kv within the token-shard group
    arin = cx.dram.tile([128, H * 32], F32, name="arin", tag="arin")
    arout = cx.dram.tile([128, H * 32], F32, name="arout", tag="arout")
    nc.sync.dma_start(arin[:], kvpack[:])
    nc.gpsimd.collective_compute(
        "AllReduce", ALU.add, replica_groups=REPLICA_GROUPS,
        ins=[arin.opt()], outs=[arout.opt()])
    # kvred trigger issued NOW so it doesn't queue behind the MLP's
    # weight-stream triggers (sync queue is in-order)
    kvred = cx.act1.tile([128, H * 32], F32, name="kvred", tag="kvred")
    nc.sync.dma_start(kvred[:], arout[:])

    # ---- prefetch wout ft-chunks for the attention GEMM
    wos = []
    for ft in range(KT):
        c = cx.stream.tile([128, KT, 128], F16, name="wos", tag="wos")
        nc.sync.dma_start(c[:], seg["wout"].ap()[ft])
        wos.append(c)

    # ---- attention block: emitted BEFORE the deferred MLP so it has
    # higher list-scheduler priority and preempts leftover MLP work the
    # moment kvred lands; the MLP (always-ready, lower priority) fills
    # the all-reduce window and h-chain stalls.
    # block-diag kv (f16, scaled by KVS; wout carries the 256x)
    for hp in range(KT):
        nc.vector.tensor_scalar_mul(cx.bd16[0:64, hp, 0:64],
                                    kvred[0:64, hp * 64:(hp + 1) * 64], KVS)
        nc.vector.tensor_scalar_mul(cx.bd16[64:128, hp, 64:128],
                                    kvred[64:128, hp * 64:(hp + 1) * 64], KVS)
    o16 = cx.act1.tile([128, KT, TOK], F16, name="o16", tag="o16")
    for hp in range(KT):
        ps = cx.psA.tile([128, 2 * TOK], F32, name="ps", tag="ps")
        nc.tensor.matmul(ps[:, 0:TOK], cx.bd16[:, hp, :], q16[:, hp, :],
                         start=True, stop=True)
        nc.vector.tensor_copy(o16[:, hp, :], ps[:, 0:TOK])

    # attn (feature-major); (attn+bout) gathered into at32 on ScalarE,
    # then two wide DVE adds + per-ft pos STT update x2 and heff
    at32 = cx.act1.tile([128, KT, TOK], F32, name="at32", tag="at32")
    for ft in range(KT):
        ps = cx.psA.tile([128, 2 * TOK], F32, name="ps", tag="ps")
        for hp in range(KT):
            nc.tensor.matmul(ps[:, 0:TOK], wos[ft][:, hp, :],
                             o16[:, hp, :], start=(hp == 0), stop=(hp == KT - 1))
        nc.scalar.activation(at32[:, ft, :], ps[:, 0:TOK], AF.Identity,
                             bias=w["bout"][:, ft:ft + 1])
    # x2 = attn + x_eff, in place over xeff (must read at32 before the
    # heff update below overwrites it)
    nc.vector.tensor_add(xeff[:], at32[:], xeff[:])
    if nxt is not None:
        nc.vector.tensor_add(at32[:], at32[:], heff[:])
        for ft in range(KT):
            nc.vector.scalar_tensor_tensor(
                heff[:, ft, :], cx.spat[:, ft, :], cx.tpos[:, ft, tpn:tpn + 1],
                at32[:, ft, :], op0=ALU.mult, op1=ALU.add)

    # ---- next frame's x-side (fills the all-reduce latency)
    xs_next = _x_stage(nc, cx, x_src, nxt) if nxt is not None else None

    # ---- deferred MLP of the previous frame (hides the all-reduce)
    if pend is not None:
        _emit_mlp(nc, cx, seg, w, pend)

    # ---- output LN -> z2 for the deferred MLP
    ob, rbo, meano, rb32o = _ln_chain(nc, cx, xeff, "o")
    R2 = cx.act2.tile([33, TOK], F16, name="R2", tag="R2")
    nc.vector.memset(R2[:], 0.0)     # garbage rows x zero weights else NaN
    nc.vector.memset(R2[0:1, :], 1.0)
    nc.vector.tensor_mul(R2[32:33, :], meano[32:33, :], rb32o[32:33, :])
    z2 = _normalize(nc, cx, cx.act2, ob, rbo, "o")

    pend = dict(t=t, z2=z2, R2=R2, x232=xeff, fwd=fwd, out_dst=out_dst,
                yf_sc=yf_sc)
    return pend, xs_next


def _emit_mlp(nc, cx, seg, w, pend):
    t, z2, R2, x232 = pend["t"], pend["z2"], pend["R2"], pend["x232"]
    fwd, out_dst, yf_sc = pend["fwd"], pend["out_dst"], pend["yf_sc"]

    # y2 accumulators pair two ft per PSUM bank (3 banks total)
    yps = [cx.psY.tile([128, 2 * TOK], F32, name="psy", tag="psy")
           for _ in range(KT // 2)]

    def ypsl(ft):
        return yps[ft // 2][:, (ft % 2) * TOK:(ft % 2 + 1) * TOK]

    for mj in range(MT):
        # bulk weight streams ride the gpsimd SW-DGE queue so their
        # slot-waits never block the sync queue's latency DMAs
        w2s = cx.stream.tile([128, D], F16, name="w2s", tag="w2s")
        nc.gpsimd.dma_start(w2s[:], seg["w2"].ap()[mj])
        if mj < RES_MJ:
            g1sl = (lambda kd, mj=mj: w["g1"][:, mj, kd, :])
        else:
            g1t = cx.stream.tile([128, KT, 128], F16, name="g1s", tag="g1s")
            nc.gpsimd.dma_start(g1t[:], seg["g1s"].ap()[mj - RES_MJ])
            g1sl = (lambda kd, g1t=g1t: g1t[:, kd, :])
        ps = cx.psA.tile([128, 2 * TOK], F32, name="ps", tag="ps")
        for kd in range(KT):
            nc.tensor.matmul(ps[:, 0:TOK], g1sl(kd), z2[:, kd, :],
                             start=(kd == 0), stop=False)
        nc.tensor.matmul(ps[:, 0:TOK], w["B2"][:, mj * 128:(mj + 1) * 128],
                         R2[:], start=False, stop=True)
        y1c = cx.stream.tile([128, TOK], F16, name="y1c", tag="y1c")
        nc.scalar.activation(y1c[:], ps[:, 0:TOK], AF.Gelu)
        for ft in range(KT):
            nc.tensor.matmul(ypsl(ft), w2s[:, ft * 128:(ft + 1) * 128],
                             y1c[:], start=(mj == 0), stop=(mj == MT - 1))

    if fwd:
        for ft in range(KT):
            nc.vector.scalar_tensor_tensor(
                x232[:, ft, :], ypsl(ft), w["b2"][:, ft:ft + 1],
                x232[:, ft, :], op0=ALU.add, op1=ALU.add)
    else:
        yf = cx.act1.tile([128, KT, TOK], F32, name="yfld", tag="yfld")
        nc.sync.dma_start(yf[:], yf_sc[t])
        for ft in range(KT):
            yb = cx.tmp.tile([128, TOK], F32, name="yb", tag="yb")
            nc.vector.scalar_tensor_tensor(
                yb[:], ypsl(ft), w["b2"][:, ft:ft + 1], x232[:, ft, :],
                op0=ALU.add, op1=ALU.add)
            nc.vector.tensor_add(x232[:, ft, :], yb[:], yf[:, ft, :])
    nc.sync.dma_start(out_dst[t], x232[:])


# ---------------------------------------------------------------- entry point

@functools.cache
def _compiled_nc():
    return build_nc()


def kernel(**inputs):
    inputs = {k: np.asarray(v) for k, v in inputs.items()}
    nc = _compiled_nc()
    in_maps = make_in_maps(inputs)
    res = run_bass_kernel_spmd(nc, in_maps, list(range(NCORES)))
    return unshard_output(res.results)


# revision 48
# speedup vs baseline: 1.0566x; 1.0566x over previous
"""Trainium2 Bass kernel for nn_GPTrack2D (dense transformer with linear
attention and a per-frame recurrence over L).

Sharding: batch (2) -> two groups of 4 cores; tokens (1024 -> 256/core)
within each group. Linear attention's k^T v state is all-reduced per frame
within the group; the all-reduce hides behind the previous frame's MLP and
the next frame's x-side LayerNorm (software-pipelined emission).

v2 restructure vs baseline:
- All weights except w2 are SBUF-resident per scan segment (no per-frame
  g1 streaming); w2 streams in per-mj chunks with prefetch.
- LayerNorm: stats from bf16 copies via ones-matmuls into one packed PSUM
  bank; rsqrt via Ln+Exp on ScalarE (no slow DVE reciprocal); the mean
  term is folded into each GEMM as a rank-1 (K=3) matmul with host-packed
  [bias; -colsum(W); -colsum(Wh)] rows, so normalize is a single bf16
  multiply z' = bf16(x) * rsqrt(var).
- MLP mj-loop fused (y1 chunk -> gelu -> y2 accumulate), no big y1 buffer.
- heff (h + pos) is maintained directly by the attention tail, removing
  the per-frame pos re-add from the critical chain.
- Wout pre-scaled by 256 on host so the f16 kv-state scaling (1/256)
  cancels without an extra vector op.
- Emission order per frame: h-chain, qkv+collective launch, next frame's
  x-side, previous frame's MLP, attention tail, output LN.

Precision: residual stream / carry f32; matmul operands f16 except LN
stat inputs (bf16 for range: |h| reaches ~1.3e6). kv state scaled by
1/256 to fit f16 (max |m|/sd <= 0.13 so the bf16 mean-fold is safe).
"""

import functools

import numpy as np

import concourse.bacc as bacc
import concourse.mybir as mybir
from concourse import tile
from concourse.bass_utils import run_bass_kernel_spmd

F32 = mybir.dt.float32
BF16 = mybir.dt.bfloat16
F16 = mybir.dt.float16
AF = mybir.ActivationFunctionType
ALU = mybir.AluOpType

B, L, N, D, M, H = 2, 12, 1024, 768, 3072, 12
NCORES = 8
GROUP = 4                 # cores per batch group
TOK = N // GROUP          # 256 tokens per core
KT = D // 128             # 6 d-tiles
MT = M // 128             # 24 m-tiles
F3 = 3 * D                # 2304
EPS = 1e-5
KVS = 1.0 / 256.0         # kv-state scale so fp16 holds it
KVSI = 256.0

# dev-scale knobs (full problem: L_RUN=12, LAYERS_RUN=2, DIRS_RUN=(0, 1))
L_RUN = L
LAYERS_RUN = 2
DIRS_RUN = (0, 1)
RES_MJ = 12               # g1 m-tiles resident in SBUF; the rest stream

REPLICA_GROUPS = [[0, 1, 2, 3], [4, 5, 6, 7]]


# ---------------------------------------------------------------- host prep

def _pack_weights(inputs, dtype=np.float16):
    segs = []
    for layer in range(LAYERS_RUN):
        for d in DIRS_RUN:
            gi = np.asarray(inputs["lni_g"][d, layer]); bi = np.asarray(inputs["lni_b"][d, layer])
            gh = np.asarray(inputs["lnh_g"][d, layer]); bh = np.asarray(inputs["lnh_b"][d, layer])
            go = np.asarray(inputs["lno_g"][d, layer]); bo = np.asarray(inputs["lno_b"][d, layer])
            Wqkv = np.asarray(inputs["Wqkv"][d, layer]); bqkv = np.asarray(inputs["bqkv"][d, layer])
            Wqkvh = np.asarray(inputs["Wqkvh"][d, layer]); bqkvh = np.asarray(inputs["bqkvh"][d, layer])
            Wout = np.asarray(inputs["Wout"][d, layer]); bout = np.asarray(inputs["bout"][d, layer])
            W1 = np.asarray(inputs["W1"][d, layer]); b1 = np.asarray(inputs["b1"][d, layer])
            W2 = np.asarray(inputs["W2"][d, layer]); b2 = np.asarray(inputs["b2"][d, layer])

            gqkv = gi[:, None] * Wqkv                      # (D, 3D)
            gqkvh = gh[:, None] * Wqkvh
            cqkv = bi @ Wqkv + bqkv + bh @ Wqkvh + bqkvh   # (3D,)
            # rows live on partitions 0/32/64 (DVE base-partition rule);
            # the zero rows annihilate whatever sits in the R3 tile between
            B3 = np.zeros((65, F3), np.float32)
            B3[0], B3[32], B3[64] = cqkv, -gqkv.sum(0), -gqkvh.sum(0)
            g1 = go[:, None] * W1                          # (D, M)
            c1 = bo @ W1 + b1                              # (M,)
            B2 = np.zeros((33, M), np.float32)
            B2[0], B2[32] = c1, -g1.sum(0)

            seg = dict(
                # (128, KT, F3): [p, kd, f] = gqkv[kd*128+p, f]
                gqkv=np.ascontiguousarray(
                    gqkv.reshape(KT, 128, F3).transpose(1, 0, 2)).astype(dtype),
                gqkvh=np.ascontiguousarray(
                    gqkvh.reshape(KT, 128, F3).transpose(1, 0, 2)).astype(dtype),
                B3=B3.astype(dtype),
                # wout pre-scaled by KVSI: cancels the f16 kv-state 1/256.
                # ft-chunked for streaming: [ft, p, kd, c] = w[kd*128+p, ft*128+c]
                wout=np.ascontiguousarray(
                    (Wout * KVSI).reshape(KT, 128, KT, 128)
                    .transpose(2, 1, 0, 3)).astype(dtype),
                bout=np.ascontiguousarray(
                    bout.reshape(KT, 128).T).astype(np.float32),
                # resident half: (128, RES_MJ, KT, 128)
                g1=np.ascontiguousarray(
                    g1.reshape(KT, 128, MT, 128)
                    .transpose(1, 2, 0, 3)[:, :RES_MJ]).astype(dtype),
                # streamed half: (MT-RES_MJ, 128, KT, 128)
                g1s=np.ascontiguousarray(
                    g1.reshape(KT, 128, MT, 128)
                    .transpose(2, 1, 0, 3)[RES_MJ:]).astype(dtype),
                B2=B2.astype(dtype),
                w2=W2.reshape(MT, 128, D).astype(dtype),          # (MT, 128, D)
                b2=np.ascontiguousarray(
                    b2.reshape(KT, 128).T).astype(np.float32),    # (128, KT)
            )
            segs.append(seg)
    return segs


def _feat_major(a, dtype):
    """(..., tok, D) -> (..., 128, KT, tok) tiled feature-major."""
    t = np.moveaxis(np.asarray(a), -1, -2)                # (..., D, tok)
    shp = t.shape[:-2]
    t = t.reshape(shp + (KT, 128, t.shape[-1]))           # (..., KT, 128, tok)
    t = np.moveaxis(t, -3, -2)                            # (..., 128, KT, tok)
    return np.ascontiguousarray(t).astype(dtype)


def make_in_maps(inputs):
    segs = _pack_weights(inputs)
    in_maps = []
    for core in range(NCORES):
        b = core // GROUP
        s = (core % GROUP) * TOK
        m = {}
        m["x_in"] = _feat_major(
            np.asarray(inputs["x"])[b, :L_RUN, s:s + TOK, :], np.float32)
        m["h0_in"] = _feat_major(
            np.asarray(inputs["hidden"])[b, s:s + TOK, :], np.float32)
        m["spat"] = _feat_major(
            np.asarray(inputs["spatial_pos"])[b, s:s + TOK, :], np.float32)
        tp = np.asarray(inputs["temporal_pos"])[b, :L_RUN, :]   # (L, D)
        tp = tp.T.reshape(KT, 128, L_RUN).transpose(1, 0, 2)
        m["tpos"] = np.ascontiguousarray(tp).astype(np.float32)  # (128, KT, L)
        for si, seg in enumerate(segs):
            for k, v in seg.items():
                m[f"{k}_{si}"] = v
        in_maps.append(m)
    return in_maps


def unshard_output(results):
    out = np.empty((B, L_RUN, N, D), np.float32)
    for core in range(NCORES):
        b = core // GROUP
        s = (core % GROUP) * TOK
        o = np.asarray(results[core]["out_x"])            # (L, 128, KT, TOK)
        o = o.transpose(0, 2, 1, 3).reshape(L_RUN, D, TOK)
        out[b, :, s:s + TOK, :] = np.moveaxis(o, -1, -2)
    return out


# ---------------------------------------------------------------- kernel build

class Ctx:
    """Pools, constants and persistent tiles used during emission."""


def _ln_chain(nc, cx, src32, tag):
    """Feature-major LN for an SBUF (128, KT, TOK) f32 tile.

    Emits: bf16 copy xb, squares, packed stats matmuls (s1|s2 in one
    PSUM bank), mean/var smalls, rsqrt via Ln+Exp. Returns (xb, rbb,
    mrow, rb32): xb bf16 copy, rbb bf16 (128,TOK) rsqrt, plus the f32
    mean/rb tiles for the mrb row write.
    """
    # xq packs [bf16 copy | its square] so one 512-wide matmul per kd
    # yields both stat sums; squares on DVE keep ScalarE's LUT unthrashed
    xq = cx.act1.tile([128, KT, 2 * TOK], BF16, name=f"xb_{tag}", tag=f"xb_{tag}")
    s12 = cx.psS.tile([128, 2 * TOK], F32, name="s12", tag="s12")
    for kd in range(KT):
        nc.vector.tensor_copy(xq[:, kd, 0:TOK], src32[:, kd, :])
        nc.vector.tensor_mul(xq[:, kd, TOK:2 * TOK], xq[:, kd, 0:TOK],
                             xq[:, kd, 0:TOK])
        nc.tensor.matmul(s12[:], cx.onesB[:], xq[:, kd, :],
                         start=(kd == 0), stop=(kd == KT - 1))
    mean = cx.sm.tile([128, TOK], F32, name="mean", tag="lnsm")
    nc.vector.tensor_scalar_mul(mean[:], s12[:, 0:TOK], 1.0 / D)
    msq = cx.sm.tile([128, TOK], F32, name="msq", tag="lnsm")
    nc.vector.tensor_mul(msq[:], mean[:], mean[:])
    ve = cx.sm.tile([128, TOK], F32, name="ve", tag="lnsm")
    nc.vector.scalar_tensor_tensor(ve[:], s12[:, TOK:2 * TOK], 1.0 / D, msq[:],
                                   op0=ALU.mult, op1=ALU.subtract)
    sd = cx.sm.tile([128, TOK], F32, name="sd", tag="lnsm")
    nc.scalar.activation(sd[:], ve[:], AF.Sqrt, bias=cx.epsc[:])
    rb32 = cx.sm.tile([128, TOK], F32, name="rb32", tag="lnsm")
    nc.vector.reciprocal(rb32[:], sd[:])
    rbb = cx.tmp.tile([128, TOK], BF16, name=f"rbb_{tag}", tag=f"rbb_{tag}")
    nc.vector.tensor_copy(rbb[:], rb32[:])
    return xq, rbb, mean, rb32


def _normalize(nc, cx, pool, xq, rbb, tag, bufs_tag=None):
    """z'[kd] = xb[kd] * rbb -> f16 (128, KT, TOK)."""
    z = pool.tile([128, KT, TOK], F16, name=f"z_{tag}",
                  tag=bufs_tag or f"z_{tag}")
    for kd in range(KT):
        nc.vector.tensor_mul(z[:, kd, :], xq[:, kd, 0:TOK], rbb[:])
    return z


def _elu1(nc, cx, psum_ap, out_ap, ncols):
    """out = elu(psum)+1 = exp(min(x,0)) + max(x,0)."""
    tmin = cx.act1.tile([128, 512], F32, name="emin", tag="emin")
    texp = cx.act1.tile([128, 512], F32, name="eexp", tag="eexp")
    nc.vector.tensor_scalar_min(tmin[:, :ncols], psum_ap, 0.0)
    nc.scalar.activation(texp[:, :ncols], tmin[:, :ncols], AF.Exp)
    nc.vector.scalar_tensor_tensor(out_ap, psum_ap, 0.0, texp[:, :ncols],
                                   op0=ALU.max, op1=ALU.add)


def build_nc():
    nc = bacc.Bacc("TRN2", target_bir_lowering=False, debug=False,
                   num_devices=NCORES)

    x_in = nc.dram_tensor("x_in", [L_RUN, 128, KT, TOK], F32, kind="ExternalInput")
    h0_in = nc.dram_tensor("h0_in", [128, KT, TOK], F32, kind="ExternalInput")
    spat = nc.dram_tensor("spat", [128, KT, TOK], F32, kind="ExternalInput")
    tpos = nc.dram_tensor("tpos", [128, KT, L_RUN], F32, kind="ExternalInput")
    nseg = LAYERS_RUN * len(DIRS_RUN)
    segs = []
    for si in range(nseg):
        segs.append(dict(
            gqkv=nc.dram_tensor(f"gqkv_{si}", [128, KT, F3], F16, kind="ExternalInput"),
            gqkvh=nc.dram_tensor(f"gqkvh_{si}", [128, KT, F3], F16, kind="ExternalInput"),
            B3=nc.dram_tensor(f"B3_{si}", [65, F3], F16, kind="ExternalInput"),
            wout=nc.dram_tensor(f"wout_{si}", [KT, 128, KT, 128], F16, kind="ExternalInput"),
            bout=nc.dram_tensor(f"bout_{si}", [128, KT], F32, kind="ExternalInput"),
            g1=nc.dram_tensor(f"g1_{si}", [128, RES_MJ, KT, 128], F16, kind="ExternalInput"),
            g1s=nc.dram_tensor(f"g1s_{si}", [MT - RES_MJ, 128, KT, 128], F16, kind="ExternalInput"),
            B2=nc.dram_tensor(f"B2_{si}", [33, M], F16, kind="ExternalInput"),
            w2=nc.dram_tensor(f"w2_{si}", [MT, 128, D], F16, kind="ExternalInput"),
            b2=nc.dram_tensor(f"b2_{si}", [128, KT], F32, kind="ExternalInput"),
        ))
    out_x = nc.dram_tensor("out_x", [L_RUN, 128, KT, TOK], F32, kind="ExternalOutput")

    with tile.TileContext(nc) as tc:
        with (
            tc.tile_pool(name="cst", bufs=1) as cst,
            tc.tile_pool(name="wt", bufs=1) as wt,
            tc.tile_pool(name="stream", bufs=3) as stream,
            tc.tile_pool(name="act1", bufs=1) as act1,
            tc.tile_pool(name="act2", bufs=2) as act2,
            tc.tile_pool(name="state", bufs=1) as state,
            tc.tile_pool(name="tmp", bufs=2) as tmp,
            tc.tile_pool(name="sm", bufs=6) as sm,
            tc.tile_pool(name="psA", bufs=2, space="PSUM") as psA,
            tc.tile_pool(name="psS", bufs=1, space="PSUM") as psS,
            tc.tile_pool(name="psM", bufs=2, space="PSUM") as psM,
            tc.tile_pool(name="psY", bufs=3, space="PSUM") as psY,
            tc.tile_pool(name="dram", bufs=2, space="DRAM") as dram,
        ):
            cx = Ctx()
            cx.wt, cx.stream, cx.act1, cx.act2 = wt, stream, act1, act2
            cx.state, cx.tmp, cx.sm = state, tmp, sm
            cx.psA, cx.psS, cx.psM = psA, psS, psM
            cx.psY, cx.dram = psY, dram

            cx.onesB = cst.tile([128, 128], BF16, name="onesB")
            nc.vector.memset(cx.onesB[:], 1.0)
            cx.epsc = cst.tile([128, 1], F32, name="epsc")
            nc.vector.memset(cx.epsc[:], EPS)
            cx.spat = cst.tile([128, KT, TOK], F32, name="spatc")
            nc.sync.dma_start(cx.spat[:], spat.ap())
            cx.tpos = cst.tile([128, KT, L_RUN], F32, name="tposc")
            nc.sync.dma_start(cx.tpos[:], tpos.ap())
            # block-diag kv holder: off-diagonal blocks stay zero forever
            cx.bd16 = state.tile([128, KT, 128], F16, name="bd16", tag="bd16")
            nc.vector.memset(cx.bd16[:], 0.0)
            cx.h0_in = h0_in

            x1_sc = dram.tile([L_RUN, 128, KT, TOK], F32, name="x1_sc", tag="x1_sc")
            yf_sc = dram.tile([L_RUN, 128, KT, TOK], F32, name="yf_sc", tag="yf_sc")

            for layer in range(LAYERS_RUN):
                x_src = x_in.ap() if layer == 0 else x1_sc
                last_layer = layer == LAYERS_RUN - 1
                for dir_i, d in enumerate(DIRS_RUN):
                    si = layer * len(DIRS_RUN) + dir_i
                    fwd = d == 0
                    last_scan = dir_i == len(DIRS_RUN) - 1
                    frames = (list(range(L_RUN)) if fwd
                              else list(range(L_RUN - 1, -1, -1)))
                    if not last_scan:
                        out_dst = yf_sc
                    elif last_layer:
                        out_dst = out_x.ap()
                    else:
                        out_dst = x1_sc
                    _emit_scan(nc, cx, segs[si], x_src, frames,
                               pos_fixed=(layer if fwd else None),
                               yf_sc=yf_sc, fwd=fwd, out_dst=out_dst)
    nc.compile()
    return nc


def _emit_scan(nc, cx, seg, x_src, frames, pos_fixed, yf_sc, fwd,
               out_dst):
    w = {}
    for nm, shape, dt in (("gqkv", [128, KT, F3], F16),
                          ("gqkvh", [128, KT, F3], F16),
                          ("B3", [65, F3], F16),
                          ("g1", [128, RES_MJ, KT, 128], F16),
                          ("B2", [33, M], F16),
                          ("bout", [128, KT], F32),
                          ("b2", [128, KT], F32)):
        w[nm] = cx.wt.tile(shape, dt, name=nm, tag=nm)
        nc.sync.dma_start(w[nm][:], seg[nm].ap())

    # heff = h0 + pos[tp0] (f32 carry, maintained by the attention tail);
    # h0 borrows the yfld slot (idle at scan starts)
    h0t = cx.act1.tile([128, KT, TOK], F32, name="yfld", tag="yfld")
    nc.sync.dma_start(h0t[:], cx.h0_in.ap())
    heff = cx.state.tile([128, KT, TOK], F32, name="heff", tag="heff")
    tp0 = pos_fixed if pos_fixed is not None else frames[0]
    for kd in range(KT):
        nc.vector.scalar_tensor_tensor(
            heff[:, kd, :], cx.spat[:, kd, :], cx.tpos[:, kd, tp0:tp0 + 1],
            h0t[:, kd, :], op0=ALU.mult, op1=ALU.add)

    xs = _x_stage(nc, cx, x_src, frames[0])
    pend = None
    for i, t in enumerate(frames):
        nxt = frames[i + 1] if i + 1 < len(frames) else None
        pend, xs = _emit_frame(nc, cx, seg, w, t, nxt, x_src, heff, xs,
                               pos_fixed, yf_sc, fwd, out_dst, pend)
    _emit_mlp(nc, cx, seg, w, pend)


def _x_stage(nc, cx, x_src, t):
    """x-side work for frame t: load, add pos, LN stats, normalize.

    Returns dict(xeff, zx, R3) where R3 rows are [ones; mrb_x; <mrb_h>]
    (row 2 filled later by the h-stage).
    """
    # xeff doubles as x2 later (attn tail adds in place); lives until the
    # deferred MLP tail of this frame -> 3 buffers (stream pool)
    xeff = cx.stream.tile([128, KT, TOK], F32, name="xe", tag="xe")
    nc.sync.dma_start(xeff[:], x_src[t])
    for kd in range(KT):
        nc.vector.scalar_tensor_tensor(
            xeff[:, kd, :], cx.spat[:, kd, :], cx.tpos[:, kd, t:t + 1],
            xeff[:, kd, :], op0=ALU.mult, op1=ALU.add)
    xb, rbb, mean, rb32 = _ln_chain(nc, cx, xeff, "x")
    R3 = cx.act2.tile([65, TOK], F16, name="R3", tag="R3")
    nc.vector.memset(R3[:], 0.0)     # garbage rows x zero weights else NaN
    nc.vector.memset(R3[0:1, :], 1.0)
    # stats are partition-replicated; read partition 32 to write row 32
    nc.vector.tensor_mul(R3[32:33, :], mean[32:33, :], rb32[32:33, :])
    zx = _normalize(nc, cx, cx.act2, xb, rbb, "x")
    return dict(xeff=xeff, zx=zx, R3=R3)


def _emit_frame(nc, cx, seg, w, t, nxt, x_src, heff, xs, pos_fixed, yf_sc,
                fwd, out_dst, pend):
    tpn = pos_fixed if pos_fixed is not None else nxt   # next frame's pos idx

    xeff, zx, R3 = xs["xeff"], xs["zx"], xs["R3"]

    # ---- h-side LN (critical chain)
    hb, rbh, meanh, rb32h = _ln_chain(nc, cx, heff, "h")
    nc.vector.tensor_mul(R3[64:65, :], meanh[64:65, :], rb32h[64:65, :])
    zh = _normalize(nc, cx, cx.act1, hb, rbh, "h")

    # ---- q (feature-major)
    q16 = cx.act1.tile([128, KT, TOK], F16, name="q16", tag="q16")
    for ft in range(KT):
        ps = cx.psA.tile([128, 2 * TOK], F32, name="ps", tag="ps")
        for kd in range(KT):
            nc.tensor.matmul(ps[:, 0:TOK], w["gqkv"][:, kd, ft * 128:(ft + 1) * 128],
                             zx[:, kd, :], start=(kd == 0), stop=False)
        for kd in range(KT):
            nc.tensor.matmul(ps[:, 0:TOK], w["gqkvh"][:, kd, ft * 128:(ft + 1) * 128],
                             zh[:, kd, :], start=False, stop=False)
        nc.tensor.matmul(ps[:, 0:TOK], w["B3"][:, ft * 128:(ft + 1) * 128],
                         R3[:], start=False, stop=True)
        _elu1(nc, cx, ps[:, 0:TOK], q16[:, ft, :], TOK)

    # ---- k, v (token-major): (128, 2, D) each [tok-half, feature]
    k16 = cx.act1.tile([128, 2, D], F16, name="k16", tag="k16")
    v16 = cx.act1.tile([128, 2, D], F16, name="v16", tag="v16")
    for tok2 in range(2):
        for fc in range(3):  # chunks of 512 covering [D, 3D): k then v
            lo = D + fc * 512
            ps = cx.psA.tile([128, 2 * TOK], F32, name="ps", tag="ps")
            for kd in range(KT):
                nc.tensor.matmul(ps[:], zx[:, kd, tok2 * 128:(tok2 + 1) * 128],
                                 w["gqkv"][:, kd, lo:lo + 512],
                                 start=(kd == 0), stop=False)
            for kd in range(KT):
                nc.tensor.matmul(ps[:], zh[:, kd, tok2 * 128:(tok2 + 1) * 128],
                                 w["gqkvh"][:, kd, lo:lo + 512],
                                 start=False, stop=False)
            nc.tensor.matmul(ps[:], R3[:, tok2 * 128:(tok2 + 1) * 128],
                             w["B3"][:, lo:lo + 512], start=False, stop=True)
            off = fc * 512
            if fc == 0:
                _elu1(nc, cx, ps[:], k16[:, tok2, 0:512], 512)
            elif fc == 1:
                _elu1(nc, cx, ps[:, 0:256], k16[:, tok2, 512:768], 256)
                nc.vector.tensor_copy(v16[:, tok2, 0:256], ps[:, 256:512])
            else:
                nc.vector.tensor_copy(v16[:, tok2, 256:768], ps[:])

    # ---- kv state per head-pair; pack diag blocks into (128, 384) f32
    kvpack = cx.act1.tile([128, H * 32], F32, name="kvpack", tag="kvpack")
    for hp in range(KT):
        ps = cx.psA.tile([128, 2 * TOK], F32, name="ps", tag="ps")
        pskv = ps[:, 0:128]
        for tok2 in range(2):
            nc.tensor.matmul(pskv, k16[:, tok2, hp * 128:(hp + 1) * 128],
                             v16[:, tok2, hp * 128:(hp + 1) * 128],
                             start=(tok2 == 0), stop=(tok2 == 1))
        nc.vector.tensor_copy(kvpack[0:64, hp * 64:(hp + 1) * 64],
                              pskv[0:64, 0:64])
        nc.vector.tensor_copy(kvpack[64:128, hp * 64:(hp + 1) * 64],
                              pskv[64:128, 64:128])

    # ---- all-reduce kv within the token-shard group
    arin = cx.dram.tile([128, H * 32], F32, name="arin", tag="arin")
    arout = cx.dram.tile([128, H * 32], F32, name="arout", tag="arout")
    nc.sync.dma_start(arin[:], kvpack[:])
    nc.gpsimd.collective_compute(
        "AllReduce", ALU.add, replica_groups=REPLICA_GROUPS,
        ins=[arin.opt()], outs=[arout.opt()])
    # kvred trigger issued NOW so it doesn't queue behind the MLP's
    # weight-stream triggers (sync queue is in-order)
    kvred = cx.act1.tile([128, H * 32], F32, name="kvred", tag="kvred")
    nc.sync.dma_start(kvred[:], arout[:])

    # ---- prefetch wout ft-chunks for the attention GEMM
    wos = []
    for ft in range(KT):
        c = cx.stream.tile([128, KT, 128], F16, name="wos", tag="wos")
        nc.sync.dma_start(c[:], seg["wout"].ap()[ft])
        wos.append(c)

    # ---- attention block: emitted BEFORE the deferred MLP so it has
    # higher list-scheduler priority and preempts leftover MLP work the
    # moment kvred lands; the MLP (always-ready, lower priority) fills
    # the all-reduce window and h-chain stalls.
    # block-diag kv (f16, scaled by KVS; wout carries the 256x)
    for hp in range(KT):
        nc.vector.tensor_scalar_mul(cx.bd16[0:64, hp, 0:64],
                                    kvred[0:64, hp * 64:(hp + 1) * 64], KVS)
        nc.vector.tensor_scalar_mul(cx.bd16[64:128, hp, 64:128],
                                    kvred[64:128, hp * 64:(hp + 1) * 64], KVS)
    o16 = cx.act1.tile([128, KT, TOK], F16, name="o16", tag="o16")
    for hp in range(KT):
        ps = cx.psA.tile([128, 2 * TOK], F32, name="ps", tag="ps")
        nc.tensor.matmul(ps[:, 0:TOK], cx.bd16[:, hp, :], q16[:, hp, :],
                         start=True, stop=True)
        nc.vector.tensor_copy(o16[:, hp, :], ps[:, 0:TOK])

    # attn (feature-major); (attn+bout) gathered into at32 on ScalarE,
    # then two wide DVE adds + per-ft pos STT update x2 and heff
    at32 = cx.act1.tile([128, KT, TOK], F32, name="at32", tag="at32")
    for ft in range(KT):
        ps = cx.psA.tile([128, 2 * TOK], F32, name="ps", tag="ps")
        for hp in range(KT):
            nc.tensor.matmul(ps[:, 0:TOK], wos[ft][:, hp, :],
                             o16[:, hp, :], start=(hp == 0), stop=(hp == KT - 1))
        nc.scalar.activation(at32[:, ft, :], ps[:, 0:TOK], AF.Identity,
                             bias=w["bout"][:, ft:ft + 1])
    # x2 = attn + x_eff, in place over xeff (must read at32 before the
    # heff update below overwrites it)
    nc.vector.tensor_add(xeff[:], at32[:], xeff[:])
    if nxt is not None:
        nc.vector.tensor_add(at32[:], at32[:], heff[:])
        for ft in range(KT):
            nc.vector.scalar_tensor_tensor(
                heff[:, ft, :], cx.spat[:, ft, :], cx.tpos[:, ft, tpn:tpn + 1],
                at32[:, ft, :], op0=ALU.mult, op1=ALU.add)

    # ---- next frame's x-side (fills the all-reduce latency)
    xs_next = _x_stage(nc, cx, x_src, nxt) if nxt is not None else None

    # ---- deferred MLP of the previous frame (hides the all-reduce)
    if pend is not None:
        _emit_mlp(nc, cx, seg, w, pend)

    # ---- output LN -> z2 for the deferred MLP
    ob, rbo, meano, rb32o = _ln_chain(nc, cx, xeff, "o")
    R2 = cx.act2.tile([33, TOK], F16, name="R2", tag="R2")
    nc.vector.memset(R2[:], 0.0)     # garbage rows x zero weights else NaN
    nc.vector.memset(R2[0:1, :], 1.0)
    nc.vector.tensor_mul(R2[32:33, :], meano[32:33, :], rb32o[32:33, :])
    z2 = _normalize(nc, cx, cx.act2, ob, rbo, "o")

    pend = dict(t=t, z2=z2, R2=R2, x232=xeff, fwd=fwd, out_dst=out_dst,
                yf_sc=yf_sc)
    return pend, xs_next


def _emit_mlp(nc, cx, seg, w, pend):
    t, z2, R2, x232 = pend["t"], pend["z2"], pend["R2"], pend["x232"]
    fwd, out_dst, yf_sc = pend["fwd"], pend["out_dst"], pend["yf_sc"]

    # y2 accumulators pair two ft per PSUM bank (3 banks total)
    yps = [cx.psY.tile([128, 2 * TOK], F32, name="psy", tag="psy")
           for _ in range(KT // 2)]

    def ypsl(ft):
        return yps[ft // 2][:, (ft % 2) * TOK:(ft % 2 + 1) * TOK]

    for mj in range(MT):
        # bulk weight streams ride the gpsimd SW-DGE queue so their
        # slot-waits never block the sync queue's latency DMAs
        w2s = cx.stream.tile([128, D], F16, name="w2s", tag="w2s")
        nc.gpsimd.dma_start(w2s[:], seg["w2"].ap()[mj])
        if mj < RES_MJ:
            g1sl = (lambda kd, mj=mj: w["g1"][:, mj, kd, :])
        else:
            g1t = cx.stream.tile([128, KT, 128], F16, name="g1s", tag="g1s")
            nc.gpsimd.dma_start(g1t[:], seg["g1s"].ap()[mj - RES_MJ])
            g1sl = (lambda kd, g1t=g1t: g1t[:, kd, :])
        ps = cx.psM.tile([128, 2 * TOK], F32, name="psm", tag="psm")
        for kd in range(KT):
            nc.tensor.matmul(ps[:, 0:TOK], g1sl(kd), z2[:, kd, :],
                             start=(kd == 0), stop=False)
        nc.tensor.matmul(ps[:, 0:TOK], w["B2"][:, mj * 128:(mj + 1) * 128],
                         R2[:], start=False, stop=True)
        y1c = cx.stream.tile([128, TOK], F16, name="y1c", tag="y1c")
        nc.scalar.activation(y1c[:], ps[:, 0:TOK], AF.Gelu)
        for ft in range(KT):
            nc.tensor.matmul(ypsl(ft), w2s[:, ft * 128:(ft + 1) * 128],
                             y1c[:], start=(mj == 0), stop=(mj == MT - 1))

    if fwd:
        for ft in range(KT):
            nc.vector.scalar_tensor_tensor(
                x232[:, ft, :], ypsl(ft), w["b2"][:, ft:ft + 1],
                x232[:, ft, :], op0=ALU.add, op1=ALU.add)
    else:
        yf = cx.act1.tile([128, KT, TOK], F32, name="yfld", tag="yfld")
        nc.sync.dma_start(yf[:], yf_sc[t])
        for ft in range(KT):
            yb = cx.tmp.tile([128, TOK], F32, name="yb", tag="yb")
            nc.vector.scalar_tensor_tensor(
                yb[:], ypsl(ft), w["b2"][:, ft:ft + 1], x232[:, ft, :],
                op0=ALU.add, op1=ALU.add)
            nc.vector.tensor_add(x232[:, ft, :], yb[:], yf[:, ft, :])
    nc.sync.dma_start(out_dst[t], x232[:])


# ---------------------------------------------------------------- entry point

@functools.cache
def _compiled_nc():
    return build_nc()


def kernel(**inputs):
    inputs = {k: np.asarray(v) for k, v in inputs.items()}
    nc = _compiled_nc()
    in_maps = make_in_maps(inputs)
    res = run_bass_kernel_spmd(nc, in_maps, list(range(NCORES)))
    return unshard_output(res.results)


# revision 49
# speedup vs baseline: 1.1756x; 1.1126x over previous
"""Trainium2 Bass kernel for nn_GPTrack2D (dense transformer with linear
attention and a per-frame recurrence over L).

Sharding: batch (2) -> two groups of 4 cores; tokens (1024 -> 256/core)
within each group. Linear attention's k^T v state is all-reduced per frame
within the group; the all-reduce hides behind the previous frame's MLP and
the next frame's x-side LayerNorm (software-pipelined emission).

v2 restructure vs baseline:
- All weights except w2 are SBUF-resident per scan segment (no per-frame
  g1 streaming); w2 streams in per-mj chunks with prefetch.
- LayerNorm: stats from bf16 copies via ones-matmuls into one packed PSUM
  bank; rsqrt via Ln+Exp on ScalarE (no slow DVE reciprocal); the mean
  term is folded into each GEMM as a rank-1 (K=3) matmul with host-packed
  [bias; -colsum(W); -colsum(Wh)] rows, so normalize is a single bf16
  multiply z' = bf16(x) * rsqrt(var).
- MLP mj-loop fused (y1 chunk -> gelu -> y2 accumulate), no big y1 buffer.
- heff (h + pos) is maintained directly by the attention tail, removing
  the per-frame pos re-add from the critical chain.
- Wout pre-scaled by 256 on host so the f16 kv-state scaling (1/256)
  cancels without an extra vector op.
- Emission order per frame: h-chain, qkv+collective launch, next frame's
  x-side, previous frame's MLP, attention tail, output LN.

Precision: residual stream / carry f32; matmul operands f16 except LN
stat inputs (bf16 for range: |h| reaches ~1.3e6). kv state scaled by
1/256 to fit f16 (max |m|/sd <= 0.13 so the bf16 mean-fold is safe).
"""

import functools

import numpy as np

import concourse.bacc as bacc
import concourse.mybir as mybir
from concourse import tile
from concourse.bass_utils import run_bass_kernel_spmd

F32 = mybir.dt.float32
BF16 = mybir.dt.bfloat16
F16 = mybir.dt.float16
AF = mybir.ActivationFunctionType
ALU = mybir.AluOpType

B, L, N, D, M, H = 2, 12, 1024, 768, 3072, 12
NCORES = 8
GROUP = 4                 # cores per batch group
TOK = N // GROUP          # 256 tokens per core
KT = D // 128             # 6 d-tiles
MT = M // 128             # 24 m-tiles
F3 = 3 * D                # 2304
EPS = 1e-5
KVS = 1.0 / 256.0         # kv-state scale so fp16 holds it
KVSI = 256.0

# dev-scale knobs (full problem: L_RUN=12, LAYERS_RUN=2, DIRS_RUN=(0, 1))
L_RUN = L
LAYERS_RUN = 2
DIRS_RUN = (0, 1)
RES_MJ = 12               # g1 m-tiles resident in SBUF; the rest stream

REPLICA_GROUPS = [[0, 1, 2, 3], [4, 5, 6, 7]]


# ---------------------------------------------------------------- host prep

def _pack_weights(inputs, dtype=np.float16):
    segs = []
    for layer in range(LAYERS_RUN):
        for d in DIRS_RUN:
            gi = np.asarray(inputs["lni_g"][d, layer]); bi = np.asarray(inputs["lni_b"][d, layer])
            gh = np.asarray(inputs["lnh_g"][d, layer]); bh = np.asarray(inputs["lnh_b"][d, layer])
            go = np.asarray(inputs["lno_g"][d, layer]); bo = np.asarray(inputs["lno_b"][d, layer])
            Wqkv = np.asarray(inputs["Wqkv"][d, layer]); bqkv = np.asarray(inputs["bqkv"][d, layer])
            Wqkvh = np.asarray(inputs["Wqkvh"][d, layer]); bqkvh = np.asarray(inputs["bqkvh"][d, layer])
            Wout = np.asarray(inputs["Wout"][d, layer]); bout = np.asarray(inputs["bout"][d, layer])
            W1 = np.asarray(inputs["W1"][d, layer]); b1 = np.asarray(inputs["b1"][d, layer])
            W2 = np.asarray(inputs["W2"][d, layer]); b2 = np.asarray(inputs["b2"][d, layer])

            gqkv = gi[:, None] * Wqkv                      # (D, 3D)
            gqkvh = gh[:, None] * Wqkvh
            cqkv = bi @ Wqkv + bqkv + bh @ Wqkvh + bqkvh   # (3D,)
            # rows live on partitions 0/32/64 (DVE base-partition rule);
            # the zero rows annihilate whatever sits in the R3 tile between
            B3 = np.zeros((65, F3), np.float32)
            B3[0], B3[32], B3[64] = cqkv, -gqkv.sum(0), -gqkvh.sum(0)
            g1 = go[:, None] * W1                          # (D, M)
            c1 = bo @ W1 + b1                              # (M,)
            B2 = np.zeros((33, M), np.float32)
            B2[0], B2[32] = c1, -g1.sum(0)

            seg = dict(
                # (128, KT, F3): [p, kd, f] = gqkv[kd*128+p, f]
                gqkv=np.ascontiguousarray(
                    gqkv.reshape(KT, 128, F3).transpose(1, 0, 2)).astype(dtype),
                gqkvh=np.ascontiguousarray(
                    gqkvh.reshape(KT, 128, F3).transpose(1, 0, 2)).astype(dtype),
                B3=B3.astype(dtype),
                # wout pre-scaled by KVSI: cancels the f16 kv-state 1/256.
                # ft-chunked for streaming: [ft, p, kd, c] = w[kd*128+p, ft*128+c]
                wout=np.ascontiguousarray(
                    (Wout * KVSI).reshape(KT, 128, KT, 128)
                    .transpose(2, 1, 0, 3)).astype(dtype),
                bout=np.ascontiguousarray(
                    bout.reshape(KT, 128).T).astype(np.float32),
                # resident half: (128, RES_MJ, KT, 128)
                g1=np.ascontiguousarray(
                    g1.reshape(KT, 128, MT, 128)
                    .transpose(1, 2, 0, 3)[:, :RES_MJ]).astype(dtype),
                # streamed half: (MT-RES_MJ, 128, KT, 128)
                g1s=np.ascontiguousarray(
                    g1.reshape(KT, 128, MT, 128)
                    .transpose(2, 1, 0, 3)[RES_MJ:]).astype(dtype),
                B2=B2.astype(dtype),
                w2=W2.reshape(MT, 128, D).astype(dtype),          # (MT, 128, D)
                b2=np.ascontiguousarray(
                    b2.reshape(KT, 128).T).astype(np.float32),    # (128, KT)
            )
            segs.append(seg)
    return segs


def _feat_major(a, dtype):
    """(..., tok, D) -> (..., 128, KT, tok) tiled feature-major."""
    t = np.moveaxis(np.asarray(a), -1, -2)                # (..., D, tok)
    shp = t.shape[:-2]
    t = t.reshape(shp + (KT, 128, t.shape[-1]))           # (..., KT, 128, tok)
    t = np.moveaxis(t, -3, -2)                            # (..., 128, KT, tok)
    return np.ascontiguousarray(t).astype(dtype)


def make_in_maps(inputs):
    segs = _pack_weights(inputs)
    in_maps = []
    for core in range(NCORES):
        b = core // GROUP
        s = (core % GROUP) * TOK
        m = {}
        m["x_in"] = _feat_major(
            np.asarray(inputs["x"])[b, :L_RUN, s:s + TOK, :], np.float32)
        m["h0_in"] = _feat_major(
            np.asarray(inputs["hidden"])[b, s:s + TOK, :], np.float32)
        m["spat"] = _feat_major(
            np.asarray(inputs["spatial_pos"])[b, s:s + TOK, :], np.float32)
        tp = np.asarray(inputs["temporal_pos"])[b, :L_RUN, :]   # (L, D)
        tp = tp.T.reshape(KT, 128, L_RUN).transpose(1, 0, 2)
        m["tpos"] = np.ascontiguousarray(tp).astype(np.float32)  # (128, KT, L)
        for si, seg in enumerate(segs):
            for k, v in seg.items():
                m[f"{k}_{si}"] = v
        in_maps.append(m)
    return in_maps


def unshard_output(results):
    out = np.empty((B, L_RUN, N, D), np.float32)
    for core in range(NCORES):
        b = core // GROUP
        s = (core % GROUP) * TOK
        o = np.asarray(results[core]["out_x"])            # (L, 128, KT, TOK)
        o = o.transpose(0, 2, 1, 3).reshape(L_RUN, D, TOK)
        out[b, :, s:s + TOK, :] = np.moveaxis(o, -1, -2)
    return out


# ---------------------------------------------------------------- kernel build

class Ctx:
    """Pools, constants and persistent tiles used during emission."""


def _ln_chain(nc, cx, src32, tag):
    """Feature-major LN for an SBUF (128, KT, TOK) f32 tile.

    Emits: bf16 copy xb, squares, packed stats matmuls (s1|s2 in one
    PSUM bank), mean/var smalls, rsqrt via Ln+Exp. Returns (xb, rbb,
    mrow, rb32): xb bf16 copy, rbb bf16 (128,TOK) rsqrt, plus the f32
    mean/rb tiles for the mrb row write.
    """
    # xq packs [bf16 copy | its square] so one 512-wide matmul per kd
    # yields both stat sums; squares on DVE keep ScalarE's LUT unthrashed
    xq = cx.act1.tile([128, KT, 2 * TOK], BF16, name=f"xb_{tag}", tag=f"xb_{tag}")
    s12 = cx.psS.tile([128, 2 * TOK], F32, name="s12", tag="s12")
    for kd in range(KT):
        nc.vector.tensor_copy(xq[:, kd, 0:TOK], src32[:, kd, :])
        nc.vector.tensor_mul(xq[:, kd, TOK:2 * TOK], xq[:, kd, 0:TOK],
                             xq[:, kd, 0:TOK])
        nc.tensor.matmul(s12[:], cx.onesB[:], xq[:, kd, :],
                         start=(kd == 0), stop=(kd == KT - 1))
    mean = cx.sm.tile([128, TOK], F32, name="mean", tag="lnsm")
    nc.vector.tensor_scalar_mul(mean[:], s12[:, 0:TOK], 1.0 / D)
    msq = cx.sm.tile([128, TOK], F32, name="msq", tag="lnsm")
    nc.vector.tensor_mul(msq[:], mean[:], mean[:])
    ve = cx.sm.tile([128, TOK], F32, name="ve", tag="lnsm")
    nc.vector.scalar_tensor_tensor(ve[:], s12[:, TOK:2 * TOK], 1.0 / D, msq[:],
                                   op0=ALU.mult, op1=ALU.subtract)
    sd = cx.sm.tile([128, TOK], F32, name="sd", tag="lnsm")
    nc.scalar.activation(sd[:], ve[:], AF.Sqrt, bias=cx.epsc[:])
    rb32 = cx.sm.tile([128, TOK], F32, name="rb32", tag="lnsm")
    nc.vector.reciprocal(rb32[:], sd[:])
    rbb = cx.tmp.tile([128, TOK], BF16, name=f"rbb_{tag}", tag=f"rbb_{tag}")
    nc.vector.tensor_copy(rbb[:], rb32[:])
    return xq, rbb, mean, rb32


def _normalize(nc, cx, pool, xq, rbb, tag, bufs_tag=None):
    """z'[kd] = xb[kd] * rbb -> f16 (128, KT, TOK)."""
    z = pool.tile([128, KT, TOK], F16, name=f"z_{tag}",
                  tag=bufs_tag or f"z_{tag}")
    for kd in range(KT):
        nc.vector.tensor_mul(z[:, kd, :], xq[:, kd, 0:TOK], rbb[:])
    return z


def _elu1(nc, cx, psum_ap, out_ap, ncols):
    """out = elu(psum)+1 = exp(min(x,0)) + max(x,0)."""
    tmin = cx.act1.tile([128, 512], F32, name="emin", tag="emin")
    texp = cx.act1.tile([128, 512], F32, name="eexp", tag="eexp")
    nc.vector.tensor_scalar_min(tmin[:, :ncols], psum_ap, 0.0)
    nc.scalar.activation(texp[:, :ncols], tmin[:, :ncols], AF.Exp)
    nc.vector.scalar_tensor_tensor(out_ap, psum_ap, 0.0, texp[:, :ncols],
                                   op0=ALU.max, op1=ALU.add)


def build_nc():
    nc = bacc.Bacc("TRN2", target_bir_lowering=False, debug=False,
                   num_devices=NCORES)

    x_in = nc.dram_tensor("x_in", [L_RUN, 128, KT, TOK], F32, kind="ExternalInput")
    h0_in = nc.dram_tensor("h0_in", [128, KT, TOK], F32, kind="ExternalInput")
    spat = nc.dram_tensor("spat", [128, KT, TOK], F32, kind="ExternalInput")
    tpos = nc.dram_tensor("tpos", [128, KT, L_RUN], F32, kind="ExternalInput")
    nseg = LAYERS_RUN * len(DIRS_RUN)
    segs = []
    for si in range(nseg):
        segs.append(dict(
            gqkv=nc.dram_tensor(f"gqkv_{si}", [128, KT, F3], F16, kind="ExternalInput"),
            gqkvh=nc.dram_tensor(f"gqkvh_{si}", [128, KT, F3], F16, kind="ExternalInput"),
            B3=nc.dram_tensor(f"B3_{si}", [65, F3], F16, kind="ExternalInput"),
            wout=nc.dram_tensor(f"wout_{si}", [KT, 128, KT, 128], F16, kind="ExternalInput"),
            bout=nc.dram_tensor(f"bout_{si}", [128, KT], F32, kind="ExternalInput"),
            g1=nc.dram_tensor(f"g1_{si}", [128, RES_MJ, KT, 128], F16, kind="ExternalInput"),
            g1s=nc.dram_tensor(f"g1s_{si}", [MT - RES_MJ, 128, KT, 128], F16, kind="ExternalInput"),
            B2=nc.dram_tensor(f"B2_{si}", [33, M], F16, kind="ExternalInput"),
            w2=nc.dram_tensor(f"w2_{si}", [MT, 128, D], F16, kind="ExternalInput"),
            b2=nc.dram_tensor(f"b2_{si}", [128, KT], F32, kind="ExternalInput"),
        ))
    out_x = nc.dram_tensor("out_x", [L_RUN, 128, KT, TOK], F32, kind="ExternalOutput")

    with tile.TileContext(nc) as tc:
        with (
            tc.tile_pool(name="cst", bufs=1) as cst,
            tc.tile_pool(name="wt", bufs=1) as wt,
            tc.tile_pool(name="stream", bufs=3) as stream,
            tc.tile_pool(name="act1", bufs=1) as act1,
            tc.tile_pool(name="act2", bufs=2) as act2,
            tc.tile_pool(name="state", bufs=1) as state,
            tc.tile_pool(name="tmp", bufs=2) as tmp,
            tc.tile_pool(name="sm", bufs=6) as sm,
            tc.tile_pool(name="psA", bufs=2, space="PSUM") as psA,
            tc.tile_pool(name="psS", bufs=1, space="PSUM") as psS,
            tc.tile_pool(name="psM", bufs=2, space="PSUM") as psM,
            tc.tile_pool(name="psY", bufs=3, space="PSUM") as psY,
            tc.tile_pool(name="dram", bufs=2, space="DRAM") as dram,
        ):
            cx = Ctx()
            cx.wt, cx.stream, cx.act1, cx.act2 = wt, stream, act1, act2
            cx.state, cx.tmp, cx.sm = state, tmp, sm
            cx.psA, cx.psS, cx.psM = psA, psS, psM
            cx.psY, cx.dram = psY, dram

            cx.onesB = cst.tile([128, 128], BF16, name="onesB")
            nc.vector.memset(cx.onesB[:], 1.0)
            cx.epsc = cst.tile([128, 1], F32, name="epsc")
            nc.vector.memset(cx.epsc[:], EPS)
            cx.spat = cst.tile([128, KT, TOK], F32, name="spatc")
            nc.sync.dma_start(cx.spat[:], spat.ap())
            cx.tpos = cst.tile([128, KT, L_RUN], F32, name="tposc")
            nc.sync.dma_start(cx.tpos[:], tpos.ap())
            # block-diag kv holder: off-diagonal blocks stay zero forever
            cx.bd16 = state.tile([128, KT, 128], F16, name="bd16", tag="bd16")
            nc.vector.memset(cx.bd16[:], 0.0)
            cx.h0_in = h0_in

            x1_sc = dram.tile([L_RUN, 128, KT, TOK], F32, name="x1_sc", tag="x1_sc")
            yf_sc = dram.tile([L_RUN, 128, KT, TOK], F32, name="yf_sc", tag="yf_sc")

            for layer in range(LAYERS_RUN):
                x_src = x_in.ap() if layer == 0 else x1_sc
                last_layer = layer == LAYERS_RUN - 1
                for dir_i, d in enumerate(DIRS_RUN):
                    si = layer * len(DIRS_RUN) + dir_i
                    fwd = d == 0
                    last_scan = dir_i == len(DIRS_RUN) - 1
                    frames = (list(range(L_RUN)) if fwd
                              else list(range(L_RUN - 1, -1, -1)))
                    if not last_scan:
                        out_dst = yf_sc
                    elif last_layer:
                        out_dst = out_x.ap()
                    else:
                        out_dst = x1_sc
                    _emit_scan(nc, cx, segs[si], x_src, frames,
                               pos_fixed=(layer if fwd else None),
                               yf_sc=yf_sc, fwd=fwd, out_dst=out_dst)
    nc.compile()
    return nc


def _emit_scan(nc, cx, seg, x_src, frames, pos_fixed, yf_sc, fwd,
               out_dst):
    w = {}
    for nm, shape, dt in (("gqkv", [128, KT, F3], F16),
                          ("gqkvh", [128, KT, F3], F16),
                          ("B3", [65, F3], F16),
                          ("g1", [128, RES_MJ, KT, 128], F16),
                          ("B2", [33, M], F16),
                          ("bout", [128, KT], F32),
                          ("b2", [128, KT], F32)):
        w[nm] = cx.wt.tile(shape, dt, name=nm, tag=nm)
        nc.sync.dma_start(w[nm][:], seg[nm].ap())

    # heff = h0 + pos[tp0] (f32 carry, maintained by the attention tail);
    # h0 borrows the yfld slot (idle at scan starts)
    h0t = cx.act1.tile([128, KT, TOK], F32, name="yfld", tag="yfld")
    nc.sync.dma_start(h0t[:], cx.h0_in.ap())
    heff = cx.state.tile([128, KT, TOK], F32, name="heff", tag="heff")
    tp0 = pos_fixed if pos_fixed is not None else frames[0]
    for kd in range(KT):
        nc.vector.scalar_tensor_tensor(
            heff[:, kd, :], cx.spat[:, kd, :], cx.tpos[:, kd, tp0:tp0 + 1],
            h0t[:, kd, :], op0=ALU.mult, op1=ALU.add)

    xs = _x_stage(nc, cx, x_src, frames[0])
    pend = None
    for i, t in enumerate(frames):
        nxt = frames[i + 1] if i + 1 < len(frames) else None
        pend, xs = _emit_frame(nc, cx, seg, w, t, nxt, x_src, heff, xs,
                               pos_fixed, yf_sc, fwd, out_dst, pend)
    _emit_mlp(nc, cx, seg, w, pend)


def _x_stage(nc, cx, x_src, t):
    """x-side work for frame t: load, add pos, LN stats, normalize.

    Returns dict(xeff, zx, R3) where R3 rows are [ones; mrb_x; <mrb_h>]
    (row 2 filled later by the h-stage).
    """
    # xeff doubles as x2 later (attn tail adds in place); lives until the
    # deferred MLP tail of this frame -> 3 buffers (stream pool)
    xeff = cx.stream.tile([128, KT, TOK], F32, name="xe", tag="xe")
    nc.sync.dma_start(xeff[:], x_src[t])
    for kd in range(KT):
        nc.vector.scalar_tensor_tensor(
            xeff[:, kd, :], cx.spat[:, kd, :], cx.tpos[:, kd, t:t + 1],
            xeff[:, kd, :], op0=ALU.mult, op1=ALU.add)
    xb, rbb, mean, rb32 = _ln_chain(nc, cx, xeff, "x")
    R3 = cx.act2.tile([65, TOK], F16, name="R3", tag="R3")
    nc.vector.memset(R3[:], 0.0)     # garbage rows x zero weights else NaN
    nc.vector.memset(R3[0:1, :], 1.0)
    # stats are partition-replicated; read partition 32 to write row 32
    nc.vector.tensor_mul(R3[32:33, :], mean[32:33, :], rb32[32:33, :])
    zx = _normalize(nc, cx, cx.act2, xb, rbb, "x")
    return dict(xeff=xeff, zx=zx, R3=R3)


def _emit_frame(nc, cx, seg, w, t, nxt, x_src, heff, xs, pos_fixed, yf_sc,
                fwd, out_dst, pend):
    tpn = pos_fixed if pos_fixed is not None else nxt   # next frame's pos idx

    xeff, zx, R3 = xs["xeff"], xs["zx"], xs["R3"]

    # ---- h-side LN (critical chain)
    hb, rbh, meanh, rb32h = _ln_chain(nc, cx, heff, "h")
    nc.vector.tensor_mul(R3[64:65, :], meanh[64:65, :], rb32h[64:65, :])
    zh = _normalize(nc, cx, cx.act1, hb, rbh, "h")

    # ---- k, v (token-major): (128, 2, D) each [tok-half, feature]
    k16 = cx.act1.tile([128, 2, D], F16, name="k16", tag="k16")
    v16 = cx.act1.tile([128, 2, D], F16, name="v16", tag="v16")
    for tok2 in range(2):
        for fc in range(3):  # chunks of 512 covering [D, 3D): k then v
            lo = D + fc * 512
            ps = cx.psA.tile([128, 2 * TOK], F32, name="ps", tag="ps")
            for kd in range(KT):
                nc.tensor.matmul(ps[:], zx[:, kd, tok2 * 128:(tok2 + 1) * 128],
                                 w["gqkv"][:, kd, lo:lo + 512],
                                 start=(kd == 0), stop=False)
            for kd in range(KT):
                nc.tensor.matmul(ps[:], zh[:, kd, tok2 * 128:(tok2 + 1) * 128],
                                 w["gqkvh"][:, kd, lo:lo + 512],
                                 start=False, stop=False)
            nc.tensor.matmul(ps[:], R3[:, tok2 * 128:(tok2 + 1) * 128],
                             w["B3"][:, lo:lo + 512], start=False, stop=True)
            off = fc * 512
            if fc == 0:
                _elu1(nc, cx, ps[:], k16[:, tok2, 0:512], 512)
            elif fc == 1:
                _elu1(nc, cx, ps[:, 0:256], k16[:, tok2, 512:768], 256)
                nc.vector.tensor_copy(v16[:, tok2, 0:256], ps[:, 256:512])
            else:
                nc.vector.tensor_copy(v16[:, tok2, 256:768], ps[:])

    # ---- kv state per head-pair; pack diag blocks into (128, 384) f32
    kvpack = cx.act1.tile([128, H * 32], F32, name="kvpack", tag="kvpack")
    for hp in range(KT):
        ps = cx.psA.tile([128, 2 * TOK], F32, name="ps", tag="ps")
        pskv = ps[:, 0:128]
        for tok2 in range(2):
            nc.tensor.matmul(pskv, k16[:, tok2, hp * 128:(hp + 1) * 128],
                             v16[:, tok2, hp * 128:(hp + 1) * 128],
                             start=(tok2 == 0), stop=(tok2 == 1))
        nc.vector.tensor_copy(kvpack[0:64, hp * 64:(hp + 1) * 64],
                              pskv[0:64, 0:64])
        nc.vector.tensor_copy(kvpack[64:128, hp * 64:(hp + 1) * 64],
                              pskv[64:128, 64:128])

    # ---- all-reduce kv within the token-shard group
    arin = cx.dram.tile([128, H * 32], F32, name="arin", tag="arin")
    arout = cx.dram.tile([128, H * 32], F32, name="arout", tag="arout")
    nc.sync.dma_start(arin[:], kvpack[:])
    nc.gpsimd.collective_compute(
        "AllReduce", ALU.add, replica_groups=REPLICA_GROUPS,
        ins=[arin.opt()], outs=[arout.opt()])
    # kvred trigger issued NOW so it doesn't queue behind the MLP's
    # weight-stream triggers (sync queue is in-order)
    kvred = cx.act1.tile([128, H * 32], F32, name="kvred", tag="kvred")
    nc.sync.dma_start(kvred[:], arout[:])

    # ---- q (feature-major): only needed after kvred, so emitted after
    # the collective launch -- its ~10us of matmuls fill the all-reduce
    # window with real work
    q16 = cx.act1.tile([128, KT, TOK], F16, name="q16", tag="q16")
    for ft in range(KT):
        ps = cx.psA.tile([128, 2 * TOK], F32, name="ps", tag="ps")
        for kd in range(KT):
            nc.tensor.matmul(ps[:, 0:TOK], w["gqkv"][:, kd, ft * 128:(ft + 1) * 128],
                             zx[:, kd, :], start=(kd == 0), stop=False)
        for kd in range(KT):
            nc.tensor.matmul(ps[:, 0:TOK], w["gqkvh"][:, kd, ft * 128:(ft + 1) * 128],
                             zh[:, kd, :], start=False, stop=False)
        nc.tensor.matmul(ps[:, 0:TOK], w["B3"][:, ft * 128:(ft + 1) * 128],
                         R3[:], start=False, stop=True)
        _elu1(nc, cx, ps[:, 0:TOK], q16[:, ft, :], TOK)

    # ---- prefetch wout ft-chunks for the attention GEMM
    wos = []
    for ft in range(KT):
        c = cx.stream.tile([128, KT, 128], F16, name="wos", tag="wos")
        nc.sync.dma_start(c[:], seg["wout"].ap()[ft])
        wos.append(c)

    # ---- attention block: emitted BEFORE the deferred MLP so it has
    # higher list-scheduler priority and preempts leftover MLP work the
    # moment kvred lands; the MLP (always-ready, lower priority) fills
    # the all-reduce window and h-chain stalls.
    # block-diag kv (f16, scaled by KVS; wout carries the 256x)
    for hp in range(KT):
        nc.vector.tensor_scalar_mul(cx.bd16[0:64, hp, 0:64],
                                    kvred[0:64, hp * 64:(hp + 1) * 64], KVS)
        nc.vector.tensor_scalar_mul(cx.bd16[64:128, hp, 64:128],
                                    kvred[64:128, hp * 64:(hp + 1) * 64], KVS)
    o16 = cx.act1.tile([128, KT, TOK], F16, name="o16", tag="o16")
    for hp in range(KT):
        ps = cx.psA.tile([128, 2 * TOK], F32, name="ps", tag="ps")
        nc.tensor.matmul(ps[:, 0:TOK], cx.bd16[:, hp, :], q16[:, hp, :],
                         start=True, stop=True)
        nc.vector.tensor_copy(o16[:, hp, :], ps[:, 0:TOK])

    # attn (feature-major); (attn+bout) gathered into at32 on ScalarE,
    # then two wide DVE adds + per-ft pos STT update x2 and heff
    at32 = cx.act1.tile([128, KT, TOK], F32, name="at32", tag="at32")
    for ft in range(KT):
        ps = cx.psA.tile([128, 2 * TOK], F32, name="ps", tag="ps")
        for hp in range(KT):
            nc.tensor.matmul(ps[:, 0:TOK], wos[ft][:, hp, :],
                             o16[:, hp, :], start=(hp == 0), stop=(hp == KT - 1))
        nc.scalar.activation(at32[:, ft, :], ps[:, 0:TOK], AF.Identity,
                             bias=w["bout"][:, ft:ft + 1])
    # x2 = attn + x_eff, in place over xeff (must read at32 before the
    # heff update below overwrites it)
    nc.vector.tensor_add(xeff[:], at32[:], xeff[:])
    if nxt is not None:
        nc.vector.tensor_add(at32[:], at32[:], heff[:])
        for ft in range(KT):
            nc.vector.scalar_tensor_tensor(
                heff[:, ft, :], cx.spat[:, ft, :], cx.tpos[:, ft, tpn:tpn + 1],
                at32[:, ft, :], op0=ALU.mult, op1=ALU.add)

    # ---- next frame's x-side (fills the all-reduce latency)
    xs_next = _x_stage(nc, cx, x_src, nxt) if nxt is not None else None

    # ---- deferred MLP of the previous frame (hides the all-reduce)
    if pend is not None:
        _emit_mlp(nc, cx, seg, w, pend)

    # ---- output LN -> z2 for the deferred MLP
    ob, rbo, meano, rb32o = _ln_chain(nc, cx, xeff, "o")
    R2 = cx.act2.tile([33, TOK], F16, name="R2", tag="R2")
    nc.vector.memset(R2[:], 0.0)     # garbage rows x zero weights else NaN
    nc.vector.memset(R2[0:1, :], 1.0)
    nc.vector.tensor_mul(R2[32:33, :], meano[32:33, :], rb32o[32:33, :])
    z2 = _normalize(nc, cx, cx.act2, ob, rbo, "o")

    pend = dict(t=t, z2=z2, R2=R2, x232=xeff, fwd=fwd, out_dst=out_dst,
                yf_sc=yf_sc)
    return pend, xs_next


def _emit_mlp(nc, cx, seg, w, pend):
    t, z2, R2, x232 = pend["t"], pend["z2"], pend["R2"], pend["x232"]
    fwd, out_dst, yf_sc = pend["fwd"], pend["out_dst"], pend["yf_sc"]

    # y2 accumulators pair two ft per PSUM bank (3 banks total)
    yps = [cx.psY.tile([128, 2 * TOK], F32, name="psy", tag="psy")
           for _ in range(KT // 2)]

    def ypsl(ft):
        return yps[ft // 2][:, (ft % 2) * TOK:(ft % 2 + 1) * TOK]

    for mj in range(MT):
        # bulk weight streams ride the gpsimd SW-DGE queue so their
        # slot-waits never block the sync queue's latency DMAs
        w2s = cx.stream.tile([128, D], F16, name="w2s", tag="w2s")
        nc.gpsimd.dma_start(w2s[:], seg["w2"].ap()[mj])
        if mj < RES_MJ:
            g1sl = (lambda kd, mj=mj: w["g1"][:, mj, kd, :])
        else:
            g1t = cx.stream.tile([128, KT, 128], F16, name="g1s", tag="g1s")
            nc.gpsimd.dma_start(g1t[:], seg["g1s"].ap()[mj - RES_MJ])
            g1sl = (lambda kd, g1t=g1t: g1t[:, kd, :])
        ps = cx.psM.tile([128, 2 * TOK], F32, name="psm", tag="psm")
        for kd in range(KT):
            nc.tensor.matmul(ps[:, 0:TOK], g1sl(kd), z2[:, kd, :],
                             start=(kd == 0), stop=False)
        nc.tensor.matmul(ps[:, 0:TOK], w["B2"][:, mj * 128:(mj + 1) * 128],
                         R2[:], start=False, stop=True)
        y1c = cx.stream.tile([128, TOK], F16, name="y1c", tag="y1c")
        nc.scalar.activation(y1c[:], ps[:, 0:TOK], AF.Gelu)
        for ft in range(KT):
            nc.tensor.matmul(ypsl(ft), w2s[:, ft * 128:(ft + 1) * 128],
                             y1c[:], start=(mj == 0), stop=(mj == MT - 1))

    if fwd:
        for ft in range(KT):
            nc.vector.scalar_tensor_tensor(
                x232[:, ft, :], ypsl(ft), w["b2"][:, ft:ft + 1],
                x232[:, ft, :], op0=ALU.add, op1=ALU.add)
    else:
        yf = cx.act1.tile([128, KT, TOK], F32, name="yfld", tag="yfld")
        nc.sync.dma_start(yf[:], yf_sc[t])
        for ft in range(KT):
            yb = cx.tmp.tile([128, TOK], F32, name="yb", tag="yb")
            nc.vector.scalar_tensor_tensor(
                yb[:], ypsl(ft), w["b2"][:, ft:ft + 1], x232[:, ft, :],
                op0=ALU.add, op1=ALU.add)
            nc.vector.tensor_add(x232[:, ft, :], yb[:], yf[:, ft, :])
    nc.sync.dma_start(out_dst[t], x232[:])


# ---------------------------------------------------------------- entry point

@functools.cache
def _compiled_nc():
    return build_nc()


def kernel(**inputs):
    inputs = {k: np.asarray(v) for k, v in inputs.items()}
    nc = _compiled_nc()
    in_maps = make_in_maps(inputs)
    res = run_bass_kernel_spmd(nc, in_maps, list(range(NCORES)))
    return unshard_output(res.results)


# revision 50
# speedup vs baseline: 1.2268x; 1.0436x over previous
"""Trainium2 Bass kernel for nn_GPTrack2D (dense transformer with linear
attention and a per-frame recurrence over L).

Sharding: batch (2) -> two groups of 4 cores; tokens (1024 -> 256/core)
within each group. Linear attention's k^T v state is all-reduced per frame
within the group; the all-reduce hides behind the previous frame's MLP and
the next frame's x-side LayerNorm (software-pipelined emission).

v2 restructure vs baseline:
- All weights except w2 are SBUF-resident per scan segment (no per-frame
  g1 streaming); w2 streams in per-mj chunks with prefetch.
- LayerNorm: stats from bf16 copies via ones-matmuls into one packed PSUM
  bank; rsqrt via Ln+Exp on ScalarE (no slow DVE reciprocal); the mean
  term is folded into each GEMM as a rank-1 (K=3) matmul with host-packed
  [bias; -colsum(W); -colsum(Wh)] rows, so normalize is a single bf16
  multiply z' = bf16(x) * rsqrt(var).
- MLP mj-loop fused (y1 chunk -> gelu -> y2 accumulate), no big y1 buffer.
- heff (h + pos) is maintained directly by the attention tail, removing
  the per-frame pos re-add from the critical chain.
- Wout pre-scaled by 256 on host so the f16 kv-state scaling (1/256)
  cancels without an extra vector op.
- Emission order per frame: h-chain, qkv+collective launch, next frame's
  x-side, previous frame's MLP, attention tail, output LN.

Precision: residual stream / carry f32; matmul operands f16 except LN
stat inputs (bf16 for range: |h| reaches ~1.3e6). kv state scaled by
1/256 to fit f16 (max |m|/sd <= 0.13 so the bf16 mean-fold is safe).
"""

import functools

import numpy as np

import concourse.bacc as bacc
import concourse.mybir as mybir
from concourse import tile
from concourse.bass_utils import run_bass_kernel_spmd

F32 = mybir.dt.float32
BF16 = mybir.dt.bfloat16
F16 = mybir.dt.float16
AF = mybir.ActivationFunctionType
ALU = mybir.AluOpType

B, L, N, D, M, H = 2, 12, 1024, 768, 3072, 12
NCORES = 8
GROUP = 4                 # cores per batch group
TOK = N // GROUP          # 256 tokens per core
KT = D // 128             # 6 d-tiles
MT = M // 128             # 24 m-tiles
F3 = 3 * D                # 2304
EPS = 1e-5
KVS = 1.0 / 256.0         # kv-state scale so fp16 holds it
KVSI = 256.0

# dev-scale knobs (full problem: L_RUN=12, LAYERS_RUN=2, DIRS_RUN=(0, 1))
L_RUN = L
LAYERS_RUN = 2
DIRS_RUN = (0, 1)
RES_MJ = 12               # g1 m-tiles resident in SBUF; the rest stream

REPLICA_GROUPS = [[0, 1, 2, 3], [4, 5, 6, 7]]


# ---------------------------------------------------------------- host prep

def _pack_weights(inputs, dtype=np.float16):
    segs = []
    for layer in range(LAYERS_RUN):
        for d in DIRS_RUN:
            gi = np.asarray(inputs["lni_g"][d, layer]); bi = np.asarray(inputs["lni_b"][d, layer])
            gh = np.asarray(inputs["lnh_g"][d, layer]); bh = np.asarray(inputs["lnh_b"][d, layer])
            go = np.asarray(inputs["lno_g"][d, layer]); bo = np.asarray(inputs["lno_b"][d, layer])
            Wqkv = np.asarray(inputs["Wqkv"][d, layer]); bqkv = np.asarray(inputs["bqkv"][d, layer])
            Wqkvh = np.asarray(inputs["Wqkvh"][d, layer]); bqkvh = np.asarray(inputs["bqkvh"][d, layer])
            Wout = np.asarray(inputs["Wout"][d, layer]); bout = np.asarray(inputs["bout"][d, layer])
            W1 = np.asarray(inputs["W1"][d, layer]); b1 = np.asarray(inputs["b1"][d, layer])
            W2 = np.asarray(inputs["W2"][d, layer]); b2 = np.asarray(inputs["b2"][d, layer])

            gqkv = gi[:, None] * Wqkv                      # (D, 3D)
            gqkvh = gh[:, None] * Wqkvh
            cqkv = bi @ Wqkv + bqkv + bh @ Wqkvh + bqkvh   # (3D,)
            # rows live on partitions 0/32/64 (DVE base-partition rule);
            # the zero rows annihilate whatever sits in the R3 tile between
            B3 = np.zeros((65, F3), np.float32)
            B3[0], B3[32], B3[64] = cqkv, -gqkv.sum(0), -gqkvh.sum(0)
            g1 = go[:, None] * W1                          # (D, M)
            c1 = bo @ W1 + b1                              # (M,)
            B2 = np.zeros((33, M), np.float32)
            B2[0], B2[32] = c1, -g1.sum(0)

            seg = dict(
                # (128, KT, F3): [p, kd, f] = gqkv[kd*128+p, f]
                gqkv=np.ascontiguousarray(
                    gqkv.reshape(KT, 128, F3).transpose(1, 0, 2)).astype(dtype),
                gqkvh=np.ascontiguousarray(
                    gqkvh.reshape(KT, 128, F3).transpose(1, 0, 2)).astype(dtype),
                B3=B3.astype(dtype),
                # wout pre-scaled by KVSI: cancels the f16 kv-state 1/256.
                # ft-chunked for streaming: [ft, p, kd, c] = w[kd*128+p, ft*128+c]
                wout=np.ascontiguousarray(
                    (Wout * KVSI).reshape(KT, 128, KT, 128)
                    .transpose(2, 1, 0, 3)).astype(dtype),
                bout=np.ascontiguousarray(
                    bout.reshape(KT, 128).T).astype(np.float32),
                # resident half: (128, RES_MJ, KT, 128)
                g1=np.ascontiguousarray(
                    g1.reshape(KT, 128, MT, 128)
                    .transpose(1, 2, 0, 3)[:, :RES_MJ]).astype(dtype),
                # streamed half: (MT-RES_MJ, 128, KT, 128)
                g1s=np.ascontiguousarray(
                    g1.reshape(KT, 128, MT, 128)
                    .transpose(2, 1, 0, 3)[RES_MJ:]).astype(dtype),
                B2=B2.astype(dtype),
                w2=W2.reshape(MT, 128, D).astype(dtype),          # (MT, 128, D)
                b2=np.ascontiguousarray(
                    b2.reshape(KT, 128).T).astype(np.float32),    # (128, KT)
            )
            segs.append(seg)
    return segs


def _feat_major(a, dtype):
    """(..., tok, D) -> (..., 128, KT, tok) tiled feature-major."""
    t = np.moveaxis(np.asarray(a), -1, -2)                # (..., D, tok)
    shp = t.shape[:-2]
    t = t.reshape(shp + (KT, 128, t.shape[-1]))           # (..., KT, 128, tok)
    t = np.moveaxis(t, -3, -2)                            # (..., 128, KT, tok)
    return np.ascontiguousarray(t).astype(dtype)


def make_in_maps(inputs):
    segs = _pack_weights(inputs)
    in_maps = []
    for core in range(NCORES):
        b = core // GROUP
        s = (core % GROUP) * TOK
        m = {}
        m["x_in"] = _feat_major(
            np.asarray(inputs["x"])[b, :L_RUN, s:s + TOK, :], np.float32)
        m["h0_in"] = _feat_major(
            np.asarray(inputs["hidden"])[b, s:s + TOK, :], np.float32)
        m["spat"] = _feat_major(
            np.asarray(inputs["spatial_pos"])[b, s:s + TOK, :], np.float32)
        tp = np.asarray(inputs["temporal_pos"])[b, :L_RUN, :]   # (L, D)
        tp = tp.T.reshape(KT, 128, L_RUN).transpose(1, 0, 2)
        m["tpos"] = np.ascontiguousarray(tp).astype(np.float32)  # (128, KT, L)
        for si, seg in enumerate(segs):
            for k, v in seg.items():
                m[f"{k}_{si}"] = v
        in_maps.append(m)
    return in_maps


def unshard_output(results):
    out = np.empty((B, L_RUN, N, D), np.float32)
    for core in range(NCORES):
        b = core // GROUP
        s = (core % GROUP) * TOK
        o = np.asarray(results[core]["out_x"])            # (L, 128, KT, TOK)
        o = o.transpose(0, 2, 1, 3).reshape(L_RUN, D, TOK)
        out[b, :, s:s + TOK, :] = np.moveaxis(o, -1, -2)
    return out


# ---------------------------------------------------------------- kernel build

class Ctx:
    """Pools, constants and persistent tiles used during emission."""


def _ln_chain(nc, cx, src32, tag):
    """Feature-major LN for an SBUF (128, KT, TOK) f32 tile.

    Emits: bf16 copy xb, squares, packed stats matmuls (s1|s2 in one
    PSUM bank), mean/var smalls, rsqrt via Ln+Exp. Returns (xb, rbb,
    mrow, rb32): xb bf16 copy, rbb bf16 (128,TOK) rsqrt, plus the f32
    mean/rb tiles for the mrb row write.
    """
    # xq packs [bf16 copy | its square] so one 512-wide matmul per kd
    # yields both stat sums; squares on DVE keep ScalarE's LUT unthrashed
    xq = cx.act1.tile([128, KT, 2 * TOK], BF16, name=f"xb_{tag}", tag=f"xb_{tag}")
    s12 = cx.psS.tile([128, 2 * TOK], F32, name="s12", tag="s12")
    for kd in range(KT):
        nc.vector.tensor_copy(xq[:, kd, 0:TOK], src32[:, kd, :])
        nc.vector.tensor_mul(xq[:, kd, TOK:2 * TOK], xq[:, kd, 0:TOK],
                             xq[:, kd, 0:TOK])
        nc.tensor.matmul(s12[:], cx.onesB[:], xq[:, kd, :],
                         start=(kd == 0), stop=(kd == KT - 1))
    mean = cx.sm.tile([128, TOK], F32, name="mean", tag="lnsm")
    nc.vector.tensor_scalar_mul(mean[:], s12[:, 0:TOK], 1.0 / D)
    msq = cx.sm.tile([128, TOK], F32, name="msq", tag="lnsm")
    nc.vector.tensor_mul(msq[:], mean[:], mean[:])
    ve = cx.sm.tile([128, TOK], F32, name="ve", tag="lnsm")
    nc.vector.scalar_tensor_tensor(ve[:], s12[:, TOK:2 * TOK], 1.0 / D, msq[:],
                                   op0=ALU.mult, op1=ALU.subtract)
    sd = cx.sm.tile([128, TOK], F32, name="sd", tag="lnsm")
    nc.scalar.activation(sd[:], ve[:], AF.Sqrt, bias=cx.epsc[:])
    rb32 = cx.sm.tile([128, TOK], F32, name="rb32", tag="lnsm")
    nc.vector.reciprocal(rb32[:], sd[:])
    rbb = cx.tmp.tile([128, TOK], BF16, name=f"rbb_{tag}", tag=f"rbb_{tag}")
    nc.vector.tensor_copy(rbb[:], rb32[:])
    return xq, rbb, mean, rb32


def _normalize(nc, cx, pool, xq, rbb, tag, bufs_tag=None):
    """z'[kd] = xb[kd] * rbb -> f16 (128, KT, TOK)."""
    z = pool.tile([128, KT, TOK], F16, name=f"z_{tag}",
                  tag=bufs_tag or f"z_{tag}")
    for kd in range(KT):
        nc.vector.tensor_mul(z[:, kd, :], xq[:, kd, 0:TOK], rbb[:])
    return z


def _elu1(nc, cx, psum_ap, out_ap, ncols):
    """out = elu(psum)+1 = exp(min(x,0)) + max(x,0)."""
    tmin = cx.act1.tile([128, 512], F32, name="emin", tag="emin")
    texp = cx.act1.tile([128, 512], F32, name="eexp", tag="eexp")
    nc.vector.tensor_scalar_min(tmin[:, :ncols], psum_ap, 0.0)
    nc.scalar.activation(texp[:, :ncols], tmin[:, :ncols], AF.Exp)
    nc.vector.scalar_tensor_tensor(out_ap, psum_ap, 0.0, texp[:, :ncols],
                                   op0=ALU.max, op1=ALU.add)


def build_nc():
    nc = bacc.Bacc("TRN2", target_bir_lowering=False, debug=False,
                   num_devices=NCORES)

    x_in = nc.dram_tensor("x_in", [L_RUN, 128, KT, TOK], F32, kind="ExternalInput")
    h0_in = nc.dram_tensor("h0_in", [128, KT, TOK], F32, kind="ExternalInput")
    spat = nc.dram_tensor("spat", [128, KT, TOK], F32, kind="ExternalInput")
    tpos = nc.dram_tensor("tpos", [128, KT, L_RUN], F32, kind="ExternalInput")
    nseg = LAYERS_RUN * len(DIRS_RUN)
    segs = []
    for si in range(nseg):
        segs.append(dict(
            gqkv=nc.dram_tensor(f"gqkv_{si}", [128, KT, F3], F16, kind="ExternalInput"),
            gqkvh=nc.dram_tensor(f"gqkvh_{si}", [128, KT, F3], F16, kind="ExternalInput"),
            B3=nc.dram_tensor(f"B3_{si}", [65, F3], F16, kind="ExternalInput"),
            wout=nc.dram_tensor(f"wout_{si}", [KT, 128, KT, 128], F16, kind="ExternalInput"),
            bout=nc.dram_tensor(f"bout_{si}", [128, KT], F32, kind="ExternalInput"),
            g1=nc.dram_tensor(f"g1_{si}", [128, RES_MJ, KT, 128], F16, kind="ExternalInput"),
            g1s=nc.dram_tensor(f"g1s_{si}", [MT - RES_MJ, 128, KT, 128], F16, kind="ExternalInput"),
            B2=nc.dram_tensor(f"B2_{si}", [33, M], F16, kind="ExternalInput"),
            w2=nc.dram_tensor(f"w2_{si}", [MT, 128, D], F16, kind="ExternalInput"),
            b2=nc.dram_tensor(f"b2_{si}", [128, KT], F32, kind="ExternalInput"),
        ))
    out_x = nc.dram_tensor("out_x", [L_RUN, 128, KT, TOK], F32, kind="ExternalOutput")

    with tile.TileContext(nc) as tc:
        with (
            tc.tile_pool(name="cst", bufs=1) as cst,
            tc.tile_pool(name="wt", bufs=1) as wt,
            tc.tile_pool(name="stream", bufs=3) as stream,
            tc.tile_pool(name="act1", bufs=1) as act1,
            tc.tile_pool(name="act2", bufs=2) as act2,
            tc.tile_pool(name="state", bufs=1) as state,
            tc.tile_pool(name="tmp", bufs=2) as tmp,
            tc.tile_pool(name="sm", bufs=6) as sm,
            tc.tile_pool(name="psA", bufs=2, space="PSUM") as psA,
            tc.tile_pool(name="psS", bufs=1, space="PSUM") as psS,
            tc.tile_pool(name="psM", bufs=2, space="PSUM") as psM,
            tc.tile_pool(name="psY", bufs=3, space="PSUM") as psY,
            tc.tile_pool(name="dram", bufs=2, space="DRAM") as dram,
        ):
            cx = Ctx()
            cx.wt, cx.stream, cx.act1, cx.act2 = wt, stream, act1, act2
            cx.state, cx.tmp, cx.sm = state, tmp, sm
            cx.psA, cx.psS, cx.psM = psA, psS, psM
            cx.psY, cx.dram = psY, dram

            cx.onesB = cst.tile([128, 128], BF16, name="onesB")
            nc.vector.memset(cx.onesB[:], 1.0)
            cx.epsc = cst.tile([128, 1], F32, name="epsc")
            nc.vector.memset(cx.epsc[:], EPS)
            cx.spat = cst.tile([128, KT, TOK], F32, name="spatc")
            nc.sync.dma_start(cx.spat[:], spat.ap())
            cx.tpos = cst.tile([128, KT, L_RUN], F32, name="tposc")
            nc.sync.dma_start(cx.tpos[:], tpos.ap())
            # block-diag kv holder: off-diagonal blocks stay zero forever
            cx.bd16 = state.tile([128, KT, 128], F16, name="bd16", tag="bd16")
            nc.vector.memset(cx.bd16[:], 0.0)
            cx.h0_in = h0_in

            x1_sc = dram.tile([L_RUN, 128, KT, TOK], F32, name="x1_sc", tag="x1_sc")
            yf_sc = dram.tile([L_RUN, 128, KT, TOK], F32, name="yf_sc", tag="yf_sc")

            for layer in range(LAYERS_RUN):
                x_src = x_in.ap() if layer == 0 else x1_sc
                last_layer = layer == LAYERS_RUN - 1
                for dir_i, d in enumerate(DIRS_RUN):
                    si = layer * len(DIRS_RUN) + dir_i
                    fwd = d == 0
                    last_scan = dir_i == len(DIRS_RUN) - 1
                    frames = (list(range(L_RUN)) if fwd
                              else list(range(L_RUN - 1, -1, -1)))
                    if not last_scan:
                        out_dst = yf_sc
                    elif last_layer:
                        out_dst = out_x.ap()
                    else:
                        out_dst = x1_sc
                    _emit_scan(nc, cx, segs[si], x_src, frames,
                               pos_fixed=(layer if fwd else None),
                               yf_sc=yf_sc, fwd=fwd, out_dst=out_dst)
    nc.compile()
    return nc


def _emit_scan(nc, cx, seg, x_src, frames, pos_fixed, yf_sc, fwd,
               out_dst):
    w = {}
    for nm, shape, dt in (("gqkv", [128, KT, F3], F16),
                          ("gqkvh", [128, KT, F3], F16),
                          ("B3", [65, F3], F16),
                          ("g1", [128, RES_MJ, KT, 128], F16),
                          ("B2", [33, M], F16),
                          ("bout", [128, KT], F32),
                          ("b2", [128, KT], F32)):
        w[nm] = cx.wt.tile(shape, dt, name=nm, tag=nm)
        nc.sync.dma_start(w[nm][:], seg[nm].ap())

    # heff = h0 + pos[tp0] (f32 carry, maintained by the attention tail);
    # h0 borrows the yfld slot (idle at scan starts)
    h0t = cx.act1.tile([128, KT, TOK], F32, name="yfld", tag="yfld")
    nc.sync.dma_start(h0t[:], cx.h0_in.ap())
    heff = cx.state.tile([128, KT, TOK], F32, name="heff", tag="heff")
    tp0 = pos_fixed if pos_fixed is not None else frames[0]
    for kd in range(KT):
        nc.vector.scalar_tensor_tensor(
            heff[:, kd, :], cx.spat[:, kd, :], cx.tpos[:, kd, tp0:tp0 + 1],
            h0t[:, kd, :], op0=ALU.mult, op1=ALU.add)

    xs = _x_stage(nc, cx, x_src, frames[0])
    pend = None
    for i, t in enumerate(frames):
        nxt = frames[i + 1] if i + 1 < len(frames) else None
        pend, xs = _emit_frame(nc, cx, seg, w, t, nxt, x_src, heff, xs,
                               pos_fixed, yf_sc, fwd, out_dst, pend)
    _emit_mlp(nc, cx, seg, w, pend)


def _x_stage(nc, cx, x_src, t):
    """x-side work for frame t: load, add pos, LN stats, normalize.

    Returns dict(xeff, zx, R3) where R3 rows are [ones; mrb_x; <mrb_h>]
    (row 2 filled later by the h-stage).
    """
    # xeff doubles as x2 later (attn tail adds in place); lives until the
    # deferred MLP tail of this frame -> 3 buffers (stream pool)
    xeff = cx.stream.tile([128, KT, TOK], F32, name="xe", tag="xe")
    nc.sync.dma_start(xeff[:], x_src[t])
    for kd in range(KT):
        nc.vector.scalar_tensor_tensor(
            xeff[:, kd, :], cx.spat[:, kd, :], cx.tpos[:, kd, t:t + 1],
            xeff[:, kd, :], op0=ALU.mult, op1=ALU.add)
    xb, rbb, mean, rb32 = _ln_chain(nc, cx, xeff, "x")
    R3 = cx.act2.tile([65, TOK], F16, name="R3", tag="R3")
    nc.vector.memset(R3[:], 0.0)     # garbage rows x zero weights else NaN
    nc.vector.memset(R3[0:1, :], 1.0)
    # stats are partition-replicated; read partition 32 to write row 32
    nc.vector.tensor_mul(R3[32:33, :], mean[32:33, :], rb32[32:33, :])
    zx = _normalize(nc, cx, cx.act2, xb, rbb, "x")
    return dict(xeff=xeff, zx=zx, R3=R3)


def _emit_frame(nc, cx, seg, w, t, nxt, x_src, heff, xs, pos_fixed, yf_sc,
                fwd, out_dst, pend):
    tpn = pos_fixed if pos_fixed is not None else nxt   # next frame's pos idx

    xeff, zx, R3 = xs["xeff"], xs["zx"], xs["R3"]

    # ---- h-side LN (critical chain)
    hb, rbh, meanh, rb32h = _ln_chain(nc, cx, heff, "h")
    nc.vector.tensor_mul(R3[64:65, :], meanh[64:65, :], rb32h[64:65, :])
    zh = _normalize(nc, cx, cx.act1, hb, rbh, "h")

    # ---- k, v (token-major): (128, 2, D) each [tok-half, feature]
    k16 = cx.act1.tile([128, 2, D], F16, name="k16", tag="k16")
    v16 = cx.act1.tile([128, 2, D], F16, name="v16", tag="v16")
    for tok2 in range(2):
        for fc in range(3):  # chunks of 512 covering [D, 3D): k then v
            lo = D + fc * 512
            ps = cx.psA.tile([128, 2 * TOK], F32, name="ps", tag="ps")
            for kd in range(KT):
                nc.tensor.matmul(ps[:], zx[:, kd, tok2 * 128:(tok2 + 1) * 128],
                                 w["gqkv"][:, kd, lo:lo + 512],
                                 start=(kd == 0), stop=False)
            for kd in range(KT):
                nc.tensor.matmul(ps[:], zh[:, kd, tok2 * 128:(tok2 + 1) * 128],
                                 w["gqkvh"][:, kd, lo:lo + 512],
                                 start=False, stop=False)
            nc.tensor.matmul(ps[:], R3[:, tok2 * 128:(tok2 + 1) * 128],
                             w["B3"][:, lo:lo + 512], start=False, stop=True)
            off = fc * 512
            if fc == 0:
                _elu1(nc, cx, ps[:], k16[:, tok2, 0:512], 512)
            elif fc == 1:
                _elu1(nc, cx, ps[:, 0:256], k16[:, tok2, 512:768], 256)
                nc.vector.tensor_copy(v16[:, tok2, 0:256], ps[:, 256:512])
            else:
                nc.vector.tensor_copy(v16[:, tok2, 256:768], ps[:])

    # ---- kv state per head-pair; pack diag blocks into (128, 384) f32
    kvpack = cx.act1.tile([128, H * 32], F32, name="kvpack", tag="kvpack")
    for hp in range(KT):
        ps = cx.psA.tile([128, 2 * TOK], F32, name="ps", tag="ps")
        pskv = ps[:, 0:128]
        for tok2 in range(2):
            nc.tensor.matmul(pskv, k16[:, tok2, hp * 128:(hp + 1) * 128],
                             v16[:, tok2, hp * 128:(hp + 1) * 128],
                             start=(tok2 == 0), stop=(tok2 == 1))
        nc.vector.tensor_copy(kvpack[0:64, hp * 64:(hp + 1) * 64],
                              pskv[0:64, 0:64])
        nc.vector.tensor_copy(kvpack[64:128, hp * 64:(hp + 1) * 64],
                              pskv[64:128, 64:128])

    # ---- all-reduce kv within the token-shard group
    arin = cx.dram.tile([128, H * 32], F32, name="arin", tag="arin")
    arout = cx.dram.tile([128, H * 32], F32, name="arout", tag="arout")
    nc.sync.dma_start(arin[:], kvpack[:])
    nc.gpsimd.collective_compute(
        "AllReduce", ALU.add, replica_groups=REPLICA_GROUPS,
        ins=[arin.opt()], outs=[arout.opt()])
    # kvred trigger issued NOW so it doesn't queue behind the MLP's
    # weight-stream triggers (sync queue is in-order)
    kvred = cx.act1.tile([128, H * 32], F32, name="kvred", tag="kvred")
    nc.sync.dma_start(kvred[:], arout[:])

    # ---- q (feature-major): only needed after kvred, so emitted after
    # the collective launch -- its ~10us of matmuls fill the all-reduce
    # window with real work
    q16 = cx.act1.tile([128, KT, TOK], F16, name="q16", tag="q16")
    for ft in range(KT):
        ps = cx.psA.tile([128, 2 * TOK], F32, name="ps", tag="ps")
        for kd in range(KT):
            nc.tensor.matmul(ps[:, 0:TOK], w["gqkv"][:, kd, ft * 128:(ft + 1) * 128],
                             zx[:, kd, :], start=(kd == 0), stop=False)
        for kd in range(KT):
            nc.tensor.matmul(ps[:, 0:TOK], w["gqkvh"][:, kd, ft * 128:(ft + 1) * 128],
                             zh[:, kd, :], start=False, stop=False)
        nc.tensor.matmul(ps[:, 0:TOK], w["B3"][:, ft * 128:(ft + 1) * 128],
                         R3[:], start=False, stop=True)
        _elu1(nc, cx, ps[:, 0:TOK], q16[:, ft, :], TOK)

    # ---- prefetch wout ft-chunks for the attention GEMM
    wos = []
    for ft in range(KT):
        c = cx.stream.tile([128, KT, 128], F16, name="wos", tag="wos")
        nc.sync.dma_start(c[:], seg["wout"].ap()[ft])
        wos.append(c)

    # ---- attention block: emitted BEFORE the deferred MLP so it has
    # higher list-scheduler priority and preempts leftover MLP work the
    # moment kvred lands; the MLP (always-ready, lower priority) fills
    # the all-reduce window and h-chain stalls.
    # block-diag kv (f16, scaled by KVS; wout carries the 256x)
    for hp in range(KT):
        nc.vector.tensor_scalar_mul(cx.bd16[0:64, hp, 0:64],
                                    kvred[0:64, hp * 64:(hp + 1) * 64], KVS)
        nc.vector.tensor_scalar_mul(cx.bd16[64:128, hp, 64:128],
                                    kvred[64:128, hp * 64:(hp + 1) * 64], KVS)
    o16 = cx.act1.tile([128, KT, TOK], F16, name="o16", tag="o16")
    for hp in range(KT):
        ps = cx.psA.tile([128, 2 * TOK], F32, name="ps", tag="ps")
        nc.tensor.matmul(ps[:, 0:TOK], cx.bd16[:, hp, :], q16[:, hp, :],
                         start=True, stop=True)
        nc.vector.tensor_copy(o16[:, hp, :], ps[:, 0:TOK])

    # attn (feature-major); (attn+bout) gathered into at32 on ScalarE,
    # then two wide DVE adds + per-ft pos STT update x2 and heff
    at32 = cx.act1.tile([128, KT, TOK], F32, name="at32", tag="at32")
    for ft in range(KT):
        ps = cx.psA.tile([128, 2 * TOK], F32, name="ps", tag="ps")
        for hp in range(KT):
            nc.tensor.matmul(ps[:, 0:TOK], wos[ft][:, hp, :],
                             o16[:, hp, :], start=(hp == 0), stop=(hp == KT - 1))
        nc.scalar.activation(at32[:, ft, :], ps[:, 0:TOK], AF.Identity,
                             bias=w["bout"][:, ft:ft + 1])
    # x2 = attn + x_eff, in place over xeff (must read at32 before the
    # heff update below overwrites it)
    nc.vector.tensor_add(xeff[:], at32[:], xeff[:])
    if nxt is not None:
        nc.vector.tensor_add(at32[:], at32[:], heff[:])
        for ft in range(KT):
            nc.vector.scalar_tensor_tensor(
                heff[:, ft, :], cx.spat[:, ft, :], cx.tpos[:, ft, tpn:tpn + 1],
                at32[:, ft, :], op0=ALU.mult, op1=ALU.add)

    # ---- output LN -> z2 for the deferred MLP; emitted right after the
    # attention tail so MLP(t) (which fills frame t+1's h-chain stall)
    # becomes ready as early as possible
    ob, rbo, meano, rb32o = _ln_chain(nc, cx, xeff, "o")
    R2 = cx.act2.tile([33, TOK], F16, name="R2", tag="R2")
    nc.vector.memset(R2[:], 0.0)     # garbage rows x zero weights else NaN
    nc.vector.memset(R2[0:1, :], 1.0)
    nc.vector.tensor_mul(R2[32:33, :], meano[32:33, :], rb32o[32:33, :])
    z2 = _normalize(nc, cx, cx.act2, ob, rbo, "o")

    # ---- next frame's x-side (fills the all-reduce latency)
    xs_next = _x_stage(nc, cx, x_src, nxt) if nxt is not None else None

    # ---- deferred MLP of the previous frame (hides the all-reduce)
    if pend is not None:
        _emit_mlp(nc, cx, seg, w, pend)

    pend = dict(t=t, z2=z2, R2=R2, x232=xeff, fwd=fwd, out_dst=out_dst,
                yf_sc=yf_sc)
    return pend, xs_next


def _emit_mlp(nc, cx, seg, w, pend):
    t, z2, R2, x232 = pend["t"], pend["z2"], pend["R2"], pend["x232"]
    fwd, out_dst, yf_sc = pend["fwd"], pend["out_dst"], pend["yf_sc"]

    # y2 accumulators pair two ft per PSUM bank (3 banks total)
    yps = [cx.psY.tile([128, 2 * TOK], F32, name="psy", tag="psy")
           for _ in range(KT // 2)]

    def ypsl(ft):
        return yps[ft // 2][:, (ft % 2) * TOK:(ft % 2 + 1) * TOK]

    for mj in range(MT):
        # bulk weight streams ride the gpsimd SW-DGE queue so their
        # slot-waits never block the sync queue's latency DMAs
        w2s = cx.stream.tile([128, D], F16, name="w2s", tag="w2s")
        nc.gpsimd.dma_start(w2s[:], seg["w2"].ap()[mj])
        if mj < RES_MJ:
            g1sl = (lambda kd, mj=mj: w["g1"][:, mj, kd, :])
        else:
            g1t = cx.stream.tile([128, KT, 128], F16, name="g1s", tag="g1s")
            nc.gpsimd.dma_start(g1t[:], seg["g1s"].ap()[mj - RES_MJ])
            g1sl = (lambda kd, g1t=g1t: g1t[:, kd, :])
        ps = cx.psM.tile([128, 2 * TOK], F32, name="psm", tag="psm")
        for kd in range(KT):
            nc.tensor.matmul(ps[:, 0:TOK], g1sl(kd), z2[:, kd, :],
                             start=(kd == 0), stop=False)
        nc.tensor.matmul(ps[:, 0:TOK], w["B2"][:, mj * 128:(mj + 1) * 128],
                         R2[:], start=False, stop=True)
        y1c = cx.stream.tile([128, TOK], F16, name="y1c", tag="y1c")
        nc.scalar.activation(y1c[:], ps[:, 0:TOK], AF.Gelu)
        for ft in range(KT):
            nc.tensor.matmul(ypsl(ft), w2s[:, ft * 128:(ft + 1) * 128],
                             y1c[:], start=(mj == 0), stop=(mj == MT - 1))

    if fwd:
        for ft in range(KT):
            nc.vector.scalar_tensor_tensor(
                x232[:, ft, :], ypsl(ft), w["b2"][:, ft:ft + 1],
                x232[:, ft, :], op0=ALU.add, op1=ALU.add)
    else:
        yf = cx.act1.tile([128, KT, TOK], F32, name="yfld", tag="yfld")
        nc.sync.dma_start(yf[:], yf_sc[t])
        for ft in range(KT):
            yb = cx.tmp.tile([128, TOK], F32, name="yb", tag="yb")
            nc.vector.scalar_tensor_tensor(
                yb[:], ypsl(ft), w["b2"][:, ft:ft + 1], x232[:, ft, :],
                op0=ALU.add, op1=ALU.add)
            nc.vector.tensor_add(x232[:, ft, :], yb[:], yf[:, ft, :])
    nc.sync.dma_start(out_dst[t], x232[:])


# ---------------------------------------------------------------- entry point

@functools.cache
def _compiled_nc():
    return build_nc()


def kernel(**inputs):
    inputs = {k: np.asarray(v) for k, v in inputs.items()}
    nc = _compiled_nc()
    in_maps = make_in_maps(inputs)
    res = run_bass_kernel_spmd(nc, in_maps, list(range(NCORES)))
    return unshard_output(res.results)


# revision 51
# speedup vs baseline: 1.2415x; 1.0120x over previous
"""Trainium2 Bass kernel for nn_GPTrack2D (dense transformer with linear
attention and a per-frame recurrence over L).

Sharding: batch (2) -> two groups of 4 cores; tokens (1024 -> 256/core)
within each group. Linear attention's k^T v state is all-reduced per frame
within the group; the all-reduce hides behind the previous frame's MLP and
the next frame's x-side LayerNorm (software-pipelined emission).

v2 restructure vs baseline:
- All weights except w2 are SBUF-resident per scan segment (no per-frame
  g1 streaming); w2 streams in per-mj chunks with prefetch.
- LayerNorm: stats from bf16 copies via ones-matmuls into one packed PSUM
  bank; rsqrt via Ln+Exp on ScalarE (no slow DVE reciprocal); the mean
  term is folded into each GEMM as a rank-1 (K=3) matmul with host-packed
  [bias; -colsum(W); -colsum(Wh)] rows, so normalize is a single bf16
  multiply z' = bf16(x) * rsqrt(var).
- MLP mj-loop fused (y1 chunk -> gelu -> y2 accumulate), no big y1 buffer.
- heff (h + pos) is maintained directly by the attention tail, removing
  the per-frame pos re-add from the critical chain.
- Wout pre-scaled by 256 on host so the f16 kv-state scaling (1/256)
  cancels without an extra vector op.
- Emission order per frame: h-chain, qkv+collective launch, next frame's
  x-side, previous frame's MLP, attention tail, output LN.

Precision: residual stream / carry f32; matmul operands f16 except LN
stat inputs (bf16 for range: |h| reaches ~1.3e6). kv state scaled by
1/256 to fit f16 (max |m|/sd <= 0.13 so the bf16 mean-fold is safe).
"""

import functools

import numpy as np

import concourse.bacc as bacc
import concourse.mybir as mybir
from concourse import tile
from concourse.bass_utils import run_bass_kernel_spmd

F32 = mybir.dt.float32
BF16 = mybir.dt.bfloat16
F16 = mybir.dt.float16
AF = mybir.ActivationFunctionType
ALU = mybir.AluOpType

B, L, N, D, M, H = 2, 12, 1024, 768, 3072, 12
NCORES = 8
GROUP = 4                 # cores per batch group
TOK = N // GROUP          # 256 tokens per core
KT = D // 128             # 6 d-tiles
MT = M // 128             # 24 m-tiles
F3 = 3 * D                # 2304
EPS = 1e-5
KVS = 1.0 / 256.0         # kv-state scale so fp16 holds it
KVSI = 256.0

# dev-scale knobs (full problem: L_RUN=12, LAYERS_RUN=2, DIRS_RUN=(0, 1))
L_RUN = L
LAYERS_RUN = 2
DIRS_RUN = (0, 1)
RES_MJ = 12               # g1 m-tiles resident in SBUF; the rest stream

REPLICA_GROUPS = [[0, 1, 2, 3], [4, 5, 6, 7]]


# ---------------------------------------------------------------- host prep

def _pack_weights(inputs, dtype=np.float16):
    segs = []
    for layer in range(LAYERS_RUN):
        for d in DIRS_RUN:
            gi = np.asarray(inputs["lni_g"][d, layer]); bi = np.asarray(inputs["lni_b"][d, layer])
            gh = np.asarray(inputs["lnh_g"][d, layer]); bh = np.asarray(inputs["lnh_b"][d, layer])
            go = np.asarray(inputs["lno_g"][d, layer]); bo = np.asarray(inputs["lno_b"][d, layer])
            Wqkv = np.asarray(inputs["Wqkv"][d, layer]); bqkv = np.asarray(inputs["bqkv"][d, layer])
            Wqkvh = np.asarray(inputs["Wqkvh"][d, layer]); bqkvh = np.asarray(inputs["bqkvh"][d, layer])
            Wout = np.asarray(inputs["Wout"][d, layer]); bout = np.asarray(inputs["bout"][d, layer])
            W1 = np.asarray(inputs["W1"][d, layer]); b1 = np.asarray(inputs["b1"][d, layer])
            W2 = np.asarray(inputs["W2"][d, layer]); b2 = np.asarray(inputs["b2"][d, layer])

            gqkv = gi[:, None] * Wqkv                      # (D, 3D)
            gqkvh = gh[:, None] * Wqkvh
            cqkv = bi @ Wqkv + bqkv + bh @ Wqkvh + bqkvh   # (3D,)
            # rows live on partitions 0/32/64 (DVE base-partition rule);
            # the zero rows annihilate whatever sits in the R3 tile between
            B3 = np.zeros((65, F3), np.float32)
            B3[0], B3[32], B3[64] = cqkv, -gqkv.sum(0), -gqkvh.sum(0)
            g1 = go[:, None] * W1                          # (D, M)
            c1 = bo @ W1 + b1                              # (M,)
            B2 = np.zeros((33, M), np.float32)
            B2[0], B2[32] = c1, -g1.sum(0)

            seg = dict(
                # (128, KT, F3): [p, kd, f] = gqkv[kd*128+p, f]
                gqkv=np.ascontiguousarray(
                    gqkv.reshape(KT, 128, F3).transpose(1, 0, 2)).astype(dtype),
                gqkvh=np.ascontiguousarray(
                    gqkvh.reshape(KT, 128, F3).transpose(1, 0, 2)).astype(dtype),
                B3=B3.astype(dtype),
                # wout pre-scaled by KVSI: cancels the f16 kv-state 1/256.
                # ft-chunked for streaming: [ft, p, kd, c] = w[kd*128+p, ft*128+c]
                wout=np.ascontiguousarray(
                    (Wout * KVSI).reshape(KT, 128, KT, 128)
                    .transpose(2, 1, 0, 3)).astype(dtype),
                bout=np.ascontiguousarray(
                    bout.reshape(KT, 128).T).astype(np.float32),
                # resident half: (128, RES_MJ, KT, 128)
                g1=np.ascontiguousarray(
                    g1.reshape(KT, 128, MT, 128)
                    .transpose(1, 2, 0, 3)[:, :RES_MJ]).astype(dtype),
                # streamed half: (MT-RES_MJ, 128, KT, 128)
                g1s=np.ascontiguousarray(
                    g1.reshape(KT, 128, MT, 128)
                    .transpose(2, 1, 0, 3)[RES_MJ:]).astype(dtype),
                B2=B2.astype(dtype),
                w2=W2.reshape(MT, 128, D).astype(dtype),          # (MT, 128, D)
                b2=np.ascontiguousarray(
                    b2.reshape(KT, 128).T).astype(np.float32),    # (128, KT)
            )
            segs.append(seg)
    return segs


def _feat_major(a, dtype):
    """(..., tok, D) -> (..., 128, KT, tok) tiled feature-major."""
    t = np.moveaxis(np.asarray(a), -1, -2)                # (..., D, tok)
    shp = t.shape[:-2]
    t = t.reshape(shp + (KT, 128, t.shape[-1]))           # (..., KT, 128, tok)
    t = np.moveaxis(t, -3, -2)                            # (..., 128, KT, tok)
    return np.ascontiguousarray(t).astype(dtype)


def make_in_maps(inputs):
    segs = _pack_weights(inputs)
    in_maps = []
    for core in range(NCORES):
        b = core // GROUP
        s = (core % GROUP) * TOK
        m = {}
        m["x_in"] = _feat_major(
            np.asarray(inputs["x"])[b, :L_RUN, s:s + TOK, :], np.float32)
        m["h0_in"] = _feat_major(
            np.asarray(inputs["hidden"])[b, s:s + TOK, :], np.float32)
        m["spat"] = _feat_major(
            np.asarray(inputs["spatial_pos"])[b, s:s + TOK, :], np.float32)
        tp = np.asarray(inputs["temporal_pos"])[b, :L_RUN, :]   # (L, D)
        tp = tp.T.reshape(KT, 128, L_RUN).transpose(1, 0, 2)
        m["tpos"] = np.ascontiguousarray(tp).astype(np.float32)  # (128, KT, L)
        for si, seg in enumerate(segs):
            for k, v in seg.items():
                m[f"{k}_{si}"] = v
        in_maps.append(m)
    return in_maps


def unshard_output(results):
    out = np.empty((B, L_RUN, N, D), np.float32)
    for core in range(NCORES):
        b = core // GROUP
        s = (core % GROUP) * TOK
        o = np.asarray(results[core]["out_x"])            # (L, 128, KT, TOK)
        o = o.transpose(0, 2, 1, 3).reshape(L_RUN, D, TOK)
        out[b, :, s:s + TOK, :] = np.moveaxis(o, -1, -2)
    return out


# ---------------------------------------------------------------- kernel build

class Ctx:
    """Pools, constants and persistent tiles used during emission."""


def _ln_chain(nc, cx, src32, tag):
    """Feature-major LN for an SBUF (128, KT, TOK) f32 tile.

    Emits: bf16 copy xb, squares, packed stats matmuls (s1|s2 in one
    PSUM bank), mean/var smalls, rsqrt via Ln+Exp. Returns (xb, rbb,
    mrow, rb32): xb bf16 copy, rbb bf16 (128,TOK) rsqrt, plus the f32
    mean/rb tiles for the mrb row write.
    """
    # xq packs [bf16 copy | its square] so one 512-wide matmul per kd
    # yields both stat sums; squares on DVE keep ScalarE's LUT unthrashed
    xq = cx.act1.tile([128, KT, 2 * TOK], BF16, name=f"xb_{tag}", tag=f"xb_{tag}")
    s12 = cx.psS.tile([128, 2 * TOK], F32, name="s12", tag="s12")
    for kd in range(KT):
        nc.vector.tensor_copy(xq[:, kd, 0:TOK], src32[:, kd, :])
        nc.vector.tensor_mul(xq[:, kd, TOK:2 * TOK], xq[:, kd, 0:TOK],
                             xq[:, kd, 0:TOK])
        nc.tensor.matmul(s12[:], cx.onesB[:], xq[:, kd, :],
                         start=(kd == 0), stop=(kd == KT - 1))
    mean = cx.sm.tile([128, TOK], F32, name="mean", tag="lnsm")
    nc.vector.tensor_scalar_mul(mean[:], s12[:, 0:TOK], 1.0 / D)
    msq = cx.sm.tile([128, TOK], F32, name="msq", tag="lnsm")
    nc.vector.tensor_mul(msq[:], mean[:], mean[:])
    ve = cx.sm.tile([128, TOK], F32, name="ve", tag="lnsm")
    nc.vector.scalar_tensor_tensor(ve[:], s12[:, TOK:2 * TOK], 1.0 / D, msq[:],
                                   op0=ALU.mult, op1=ALU.subtract)
    sd = cx.sm.tile([128, TOK], F32, name="sd", tag="lnsm")
    nc.scalar.activation(sd[:], ve[:], AF.Sqrt, bias=cx.epsc[:])
    rb32 = cx.sm.tile([128, TOK], F32, name="rb32", tag="lnsm")
    nc.vector.reciprocal(rb32[:], sd[:])
    rbb = cx.tmp.tile([128, TOK], BF16, name=f"rbb_{tag}", tag=f"rbb_{tag}")
    nc.vector.tensor_copy(rbb[:], rb32[:])
    return xq, rbb, mean, rb32


def _normalize(nc, cx, pool, xq, rbb, tag, bufs_tag=None):
    """z'[kd] = xb[kd] * rbb -> f16 (128, KT, TOK)."""
    z = pool.tile([128, KT, TOK], F16, name=f"z_{tag}",
                  tag=bufs_tag or f"z_{tag}")
    for kd in range(KT):
        nc.vector.tensor_mul(z[:, kd, :], xq[:, kd, 0:TOK], rbb[:])
    return z


def _elu1(nc, cx, psum_ap, out_ap, ncols):
    """out = elu(psum)+1 = exp(min(x,0)) + max(x,0)."""
    tmin = cx.act1.tile([128, 512], F32, name="emin", tag="emin")
    texp = cx.act1.tile([128, 512], F32, name="eexp", tag="eexp")
    nc.vector.tensor_scalar_min(tmin[:, :ncols], psum_ap, 0.0)
    nc.scalar.activation(texp[:, :ncols], tmin[:, :ncols], AF.Exp)
    nc.vector.scalar_tensor_tensor(out_ap, psum_ap, 0.0, texp[:, :ncols],
                                   op0=ALU.max, op1=ALU.add)


def build_nc():
    nc = bacc.Bacc("TRN2", target_bir_lowering=False, debug=False,
                   num_devices=NCORES)

    x_in = nc.dram_tensor("x_in", [L_RUN, 128, KT, TOK], F32, kind="ExternalInput")
    h0_in = nc.dram_tensor("h0_in", [128, KT, TOK], F32, kind="ExternalInput")
    spat = nc.dram_tensor("spat", [128, KT, TOK], F32, kind="ExternalInput")
    tpos = nc.dram_tensor("tpos", [128, KT, L_RUN], F32, kind="ExternalInput")
    nseg = LAYERS_RUN * len(DIRS_RUN)
    segs = []
    for si in range(nseg):
        segs.append(dict(
            gqkv=nc.dram_tensor(f"gqkv_{si}", [128, KT, F3], F16, kind="ExternalInput"),
            gqkvh=nc.dram_tensor(f"gqkvh_{si}", [128, KT, F3], F16, kind="ExternalInput"),
            B3=nc.dram_tensor(f"B3_{si}", [65, F3], F16, kind="ExternalInput"),
            wout=nc.dram_tensor(f"wout_{si}", [KT, 128, KT, 128], F16, kind="ExternalInput"),
            bout=nc.dram_tensor(f"bout_{si}", [128, KT], F32, kind="ExternalInput"),
            g1=nc.dram_tensor(f"g1_{si}", [128, RES_MJ, KT, 128], F16, kind="ExternalInput"),
            g1s=nc.dram_tensor(f"g1s_{si}", [MT - RES_MJ, 128, KT, 128], F16, kind="ExternalInput"),
            B2=nc.dram_tensor(f"B2_{si}", [33, M], F16, kind="ExternalInput"),
            w2=nc.dram_tensor(f"w2_{si}", [MT, 128, D], F16, kind="ExternalInput"),
            b2=nc.dram_tensor(f"b2_{si}", [128, KT], F32, kind="ExternalInput"),
        ))
    out_x = nc.dram_tensor("out_x", [L_RUN, 128, KT, TOK], F32, kind="ExternalOutput")

    with tile.TileContext(nc) as tc:
        with (
            tc.tile_pool(name="cst", bufs=1) as cst,
            tc.tile_pool(name="wt", bufs=1) as wt,
            tc.tile_pool(name="stream", bufs=3) as stream,
            tc.tile_pool(name="act1", bufs=1) as act1,
            tc.tile_pool(name="act2", bufs=2) as act2,
            tc.tile_pool(name="state", bufs=1) as state,
            tc.tile_pool(name="tmp", bufs=2) as tmp,
            tc.tile_pool(name="sm", bufs=6) as sm,
            tc.tile_pool(name="psA", bufs=2, space="PSUM") as psA,
            tc.tile_pool(name="psS", bufs=1, space="PSUM") as psS,
            tc.tile_pool(name="psM", bufs=2, space="PSUM") as psM,
            tc.tile_pool(name="psY", bufs=3, space="PSUM") as psY,
            tc.tile_pool(name="dram", bufs=2, space="DRAM") as dram,
        ):
            cx = Ctx()
            cx.wt, cx.stream, cx.act1, cx.act2 = wt, stream, act1, act2
            cx.state, cx.tmp, cx.sm = state, tmp, sm
            cx.psA, cx.psS, cx.psM = psA, psS, psM
            cx.psY, cx.dram = psY, dram

            cx.onesB = cst.tile([128, 128], BF16, name="onesB")
            nc.vector.memset(cx.onesB[:], 1.0)
            cx.epsc = cst.tile([128, 1], F32, name="epsc")
            nc.vector.memset(cx.epsc[:], EPS)
            cx.spat = cst.tile([128, KT, TOK], F32, name="spatc")
            nc.sync.dma_start(cx.spat[:], spat.ap())
            cx.tpos = cst.tile([128, KT, L_RUN], F32, name="tposc")
            nc.sync.dma_start(cx.tpos[:], tpos.ap())
            # block-diag kv holder: off-diagonal blocks stay zero forever
            cx.bd16 = state.tile([128, KT, 128], F16, name="bd16", tag="bd16")
            nc.vector.memset(cx.bd16[:], 0.0)
            cx.h0_in = h0_in

            x1_sc = dram.tile([L_RUN, 128, KT, TOK], F32, name="x1_sc", tag="x1_sc")
            yf_sc = dram.tile([L_RUN, 128, KT, TOK], F32, name="yf_sc", tag="yf_sc")

            for layer in range(LAYERS_RUN):
                x_src = x_in.ap() if layer == 0 else x1_sc
                last_layer = layer == LAYERS_RUN - 1
                for dir_i, d in enumerate(DIRS_RUN):
                    si = layer * len(DIRS_RUN) + dir_i
                    fwd = d == 0
                    last_scan = dir_i == len(DIRS_RUN) - 1
                    frames = (list(range(L_RUN)) if fwd
                              else list(range(L_RUN - 1, -1, -1)))
                    if not last_scan:
                        out_dst = yf_sc
                    elif last_layer:
                        out_dst = out_x.ap()
                    else:
                        out_dst = x1_sc
                    _emit_scan(nc, cx, segs[si], x_src, frames,
                               pos_fixed=(layer if fwd else None),
                               yf_sc=yf_sc, fwd=fwd, out_dst=out_dst)
    nc.compile()
    return nc


def _emit_scan(nc, cx, seg, x_src, frames, pos_fixed, yf_sc, fwd,
               out_dst):
    w = {}
    for nm, shape, dt in (("gqkv", [128, KT, F3], F16),
                          ("gqkvh", [128, KT, F3], F16),
                          ("B3", [65, F3], F16),
                          ("g1", [128, RES_MJ, KT, 128], F16),
                          ("B2", [33, M], F16),
                          ("bout", [128, KT], F32),
                          ("b2", [128, KT], F32)):
        w[nm] = cx.wt.tile(shape, dt, name=nm, tag=nm)
        nc.sync.dma_start(w[nm][:], seg[nm].ap())

    # heff = h0 + pos[tp0] (f32 carry, maintained by the attention tail);
    # h0 borrows the yfld slot (idle at scan starts)
    h0t = cx.act1.tile([128, KT, TOK], F32, name="yfld", tag="yfld")
    nc.sync.dma_start(h0t[:], cx.h0_in.ap())
    heff = cx.state.tile([128, KT, TOK], F32, name="heff", tag="heff")
    tp0 = pos_fixed if pos_fixed is not None else frames[0]
    for kd in range(KT):
        nc.vector.scalar_tensor_tensor(
            heff[:, kd, :], cx.spat[:, kd, :], cx.tpos[:, kd, tp0:tp0 + 1],
            h0t[:, kd, :], op0=ALU.mult, op1=ALU.add)

    xs = _x_stage(nc, cx, x_src, frames[0])
    pend = None
    for i, t in enumerate(frames):
        nxt = frames[i + 1] if i + 1 < len(frames) else None
        pend, xs = _emit_frame(nc, cx, seg, w, t, nxt, x_src, heff, xs,
                               pos_fixed, yf_sc, fwd, out_dst, pend)
    _emit_mlp(nc, cx, seg, w, pend)


def _x_stage(nc, cx, x_src, t):
    """x-side work for frame t: load, add pos, LN stats, normalize.

    Returns dict(xeff, zx, R3) where R3 rows are [ones; mrb_x; <mrb_h>]
    (row 2 filled later by the h-stage).
    """
    # xeff doubles as x2 later (attn tail adds in place); lives until the
    # deferred MLP tail of this frame -> 3 buffers (stream pool)
    xeff = cx.stream.tile([128, KT, TOK], F32, name="xe", tag="xe")
    nc.sync.dma_start(xeff[:], x_src[t])
    for kd in range(KT):
        nc.vector.scalar_tensor_tensor(
            xeff[:, kd, :], cx.spat[:, kd, :], cx.tpos[:, kd, t:t + 1],
            xeff[:, kd, :], op0=ALU.mult, op1=ALU.add)
    xb, rbb, mean, rb32 = _ln_chain(nc, cx, xeff, "x")
    R3 = cx.act2.tile([65, TOK], F16, name="R3", tag="R3")
    nc.vector.memset(R3[:], 0.0)     # garbage rows x zero weights else NaN
    nc.vector.memset(R3[0:1, :], 1.0)
    # stats are partition-replicated; read partition 32 to write row 32
    nc.vector.tensor_mul(R3[32:33, :], mean[32:33, :], rb32[32:33, :])
    zx = _normalize(nc, cx, cx.act2, xb, rbb, "x")
    return dict(xeff=xeff, zx=zx, R3=R3)


def _emit_frame(nc, cx, seg, w, t, nxt, x_src, heff, xs, pos_fixed, yf_sc,
                fwd, out_dst, pend):
    tpn = pos_fixed if pos_fixed is not None else nxt   # next frame's pos idx

    xeff, zx, R3 = xs["xeff"], xs["zx"], xs["R3"]

    # ---- h-side LN (critical chain)
    hb, rbh, meanh, rb32h = _ln_chain(nc, cx, heff, "h")
    nc.vector.tensor_mul(R3[64:65, :], meanh[64:65, :], rb32h[64:65, :])
    zh = _normalize(nc, cx, cx.act1, hb, rbh, "h")

    # ---- k, v (token-major): (128, 2, D) each [tok-half, feature]
    k16 = cx.act1.tile([128, 2, D], F16, name="k16", tag="k16")
    v16 = cx.act1.tile([128, 2, D], F16, name="v16", tag="v16")
    for tok2 in range(2):
        for fc in range(3):  # chunks of 512 covering [D, 3D): k then v
            lo = D + fc * 512
            ps = cx.psA.tile([128, 2 * TOK], F32, name="ps", tag="ps")
            for kd in range(KT):
                nc.tensor.matmul(ps[:], zx[:, kd, tok2 * 128:(tok2 + 1) * 128],
                                 w["gqkv"][:, kd, lo:lo + 512],
                                 start=(kd == 0), stop=False)
            for kd in range(KT):
                nc.tensor.matmul(ps[:], zh[:, kd, tok2 * 128:(tok2 + 1) * 128],
                                 w["gqkvh"][:, kd, lo:lo + 512],
                                 start=False, stop=False)
            nc.tensor.matmul(ps[:], R3[:, tok2 * 128:(tok2 + 1) * 128],
                             w["B3"][:, lo:lo + 512], start=False, stop=True)
            off = fc * 512
            if fc == 0:
                _elu1(nc, cx, ps[:], k16[:, tok2, 0:512], 512)
            elif fc == 1:
                _elu1(nc, cx, ps[:, 0:256], k16[:, tok2, 512:768], 256)
                nc.scalar.activation(v16[:, tok2, 0:256], ps[:, 256:512],
                                     AF.Identity)
            else:
                nc.scalar.activation(v16[:, tok2, 256:768], ps[:], AF.Identity)

    # ---- kv state per head-pair; pack diag blocks into (128, 384) f32
    kvpack = cx.act1.tile([128, H * 32], F32, name="kvpack", tag="kvpack")
    for hp in range(KT):
        ps = cx.psA.tile([128, 2 * TOK], F32, name="ps", tag="ps")
        pskv = ps[:, 0:128]
        for tok2 in range(2):
            nc.tensor.matmul(pskv, k16[:, tok2, hp * 128:(hp + 1) * 128],
                             v16[:, tok2, hp * 128:(hp + 1) * 128],
                             start=(tok2 == 0), stop=(tok2 == 1))
        nc.scalar.activation(kvpack[0:64, hp * 64:(hp + 1) * 64],
                             pskv[0:64, 0:64], AF.Identity)
        nc.scalar.activation(kvpack[64:128, hp * 64:(hp + 1) * 64],
                             pskv[64:128, 64:128], AF.Identity)

    # ---- all-reduce kv within the token-shard group
    arin = cx.dram.tile([128, H * 32], F32, name="arin", tag="arin")
    arout = cx.dram.tile([128, H * 32], F32, name="arout", tag="arout")
    nc.sync.dma_start(arin[:], kvpack[:])
    nc.gpsimd.collective_compute(
        "AllReduce", ALU.add, replica_groups=REPLICA_GROUPS,
        ins=[arin.opt()], outs=[arout.opt()])
    # kvred trigger issued NOW so it doesn't queue behind the MLP's
    # weight-stream triggers (sync queue is in-order)
    kvred = cx.act1.tile([128, H * 32], F32, name="kvred", tag="kvred")
    nc.sync.dma_start(kvred[:], arout[:])

    # ---- q (feature-major): only needed after kvred, so emitted after
    # the collective launch -- its ~10us of matmuls fill the all-reduce
    # window with real work
    q16 = cx.act1.tile([128, KT, TOK], F16, name="q16", tag="q16")
    for ft in range(KT):
        ps = cx.psA.tile([128, 2 * TOK], F32, name="ps", tag="ps")
        for kd in range(KT):
            nc.tensor.matmul(ps[:, 0:TOK], w["gqkv"][:, kd, ft * 128:(ft + 1) * 128],
                             zx[:, kd, :], start=(kd == 0), stop=False)
        for kd in range(KT):
            nc.tensor.matmul(ps[:, 0:TOK], w["gqkvh"][:, kd, ft * 128:(ft + 1) * 128],
                             zh[:, kd, :], start=False, stop=False)
        nc.tensor.matmul(ps[:, 0:TOK], w["B3"][:, ft * 128:(ft + 1) * 128],
                         R3[:], start=False, stop=True)
        _elu1(nc, cx, ps[:, 0:TOK], q16[:, ft, :], TOK)

    # ---- prefetch wout ft-chunks for the attention GEMM
    wos = []
    for ft in range(KT):
        c = cx.stream.tile([128, KT, 128], F16, name="wos", tag="wos")
        nc.sync.dma_start(c[:], seg["wout"].ap()[ft])
        wos.append(c)

    # ---- next frame's x-side: its DVE chain fills the all-reduce window
    xs_next = _x_stage(nc, cx, x_src, nxt) if nxt is not None else None

    # ---- attention block: emitted BEFORE the deferred MLP so it has
    # higher list-scheduler priority and preempts leftover MLP work the
    # moment kvred lands; the MLP (always-ready, lower priority) fills
    # the all-reduce window and h-chain stalls.
    # block-diag kv (f16, scaled by KVS; wout carries the 256x)
    for hp in range(KT):
        nc.vector.tensor_scalar_mul(cx.bd16[0:64, hp, 0:64],
                                    kvred[0:64, hp * 64:(hp + 1) * 64], KVS)
        nc.vector.tensor_scalar_mul(cx.bd16[64:128, hp, 64:128],
                                    kvred[64:128, hp * 64:(hp + 1) * 64], KVS)
    o16 = cx.act1.tile([128, KT, TOK], F16, name="o16", tag="o16")
    for hp in range(KT):
        ps = cx.psA.tile([128, 2 * TOK], F32, name="ps", tag="ps")
        nc.tensor.matmul(ps[:, 0:TOK], cx.bd16[:, hp, :], q16[:, hp, :],
                         start=True, stop=True)
        nc.vector.tensor_copy(o16[:, hp, :], ps[:, 0:TOK])

    # attn (feature-major); (attn+bout) gathered into at32 on ScalarE,
    # then two wide DVE adds + per-ft pos STT update x2 and heff
    at32 = cx.act1.tile([128, KT, TOK], F32, name="at32", tag="at32")
    for ft in range(KT):
        ps = cx.psA.tile([128, 2 * TOK], F32, name="ps", tag="ps")
        for hp in range(KT):
            nc.tensor.matmul(ps[:, 0:TOK], wos[ft][:, hp, :],
                             o16[:, hp, :], start=(hp == 0), stop=(hp == KT - 1))
        nc.scalar.activation(at32[:, ft, :], ps[:, 0:TOK], AF.Identity,
                             bias=w["bout"][:, ft:ft + 1])
    # x2 = attn + x_eff, in place over xeff (must read at32 before the
    # heff update below overwrites it)
    nc.vector.tensor_add(xeff[:], at32[:], xeff[:])
    if nxt is not None:
        nc.vector.tensor_add(at32[:], at32[:], heff[:])
        for ft in range(KT):
            nc.vector.scalar_tensor_tensor(
                heff[:, ft, :], cx.spat[:, ft, :], cx.tpos[:, ft, tpn:tpn + 1],
                at32[:, ft, :], op0=ALU.mult, op1=ALU.add)

    # ---- output LN -> z2 for the deferred MLP; emitted right after the
    # attention tail so MLP(t) (which fills frame t+1's h-chain stall)
    # becomes ready as early as possible
    ob, rbo, meano, rb32o = _ln_chain(nc, cx, xeff, "o")
    R2 = cx.act2.tile([33, TOK], F16, name="R2", tag="R2")
    nc.vector.memset(R2[:], 0.0)     # garbage rows x zero weights else NaN
    nc.vector.memset(R2[0:1, :], 1.0)
    nc.vector.tensor_mul(R2[32:33, :], meano[32:33, :], rb32o[32:33, :])
    z2 = _normalize(nc, cx, cx.act2, ob, rbo, "o")


    # ---- deferred MLP of the previous frame (hides the all-reduce)
    if pend is not None:
        _emit_mlp(nc, cx, seg, w, pend)

    pend = dict(t=t, z2=z2, R2=R2, x232=xeff, fwd=fwd, out_dst=out_dst,
                yf_sc=yf_sc)
    return pend, xs_next


def _emit_mlp(nc, cx, seg, w, pend):
    t, z2, R2, x232 = pend["t"], pend["z2"], pend["R2"], pend["x232"]
    fwd, out_dst, yf_sc = pend["fwd"], pend["out_dst"], pend["yf_sc"]

    # y2 accumulators pair two ft per PSUM bank (3 banks total)
    yps = [cx.psY.tile([128, 2 * TOK], F32, name="psy", tag="psy")
           for _ in range(KT // 2)]

    def ypsl(ft):
        return yps[ft // 2][:, (ft % 2) * TOK:(ft % 2 + 1) * TOK]

    for mj in range(MT):
        # bulk weight streams ride the gpsimd SW-DGE queue so their
        # slot-waits never block the sync queue's latency DMAs
        w2s = cx.stream.tile([128, D], F16, name="w2s", tag="w2s")
        nc.gpsimd.dma_start(w2s[:], seg["w2"].ap()[mj])
        if mj < RES_MJ:
            g1sl = (lambda kd, mj=mj: w["g1"][:, mj, kd, :])
        else:
            g1t = cx.stream.tile([128, KT, 128], F16, name="g1s", tag="g1s")
            nc.gpsimd.dma_start(g1t[:], seg["g1s"].ap()[mj - RES_MJ])
            g1sl = (lambda kd, g1t=g1t: g1t[:, kd, :])
        ps = cx.psM.tile([128, 2 * TOK], F32, name="psm", tag="psm")
        for kd in range(KT):
            nc.tensor.matmul(ps[:, 0:TOK], g1sl(kd), z2[:, kd, :],
                             start=(kd == 0), stop=False)
        nc.tensor.matmul(ps[:, 0:TOK], w["B2"][:, mj * 128:(mj + 1) * 128],
                         R2[:], start=False, stop=True)
        y1c = cx.stream.tile([128, TOK], F16, name="y1c", tag="y1c")
        nc.scalar.activation(y1c[:], ps[:, 0:TOK], AF.Gelu)
        for ft in range(KT):
            nc.tensor.matmul(ypsl(ft), w2s[:, ft * 128:(ft + 1) * 128],
                             y1c[:], start=(mj == 0), stop=(mj == MT - 1))

    if fwd:
        for ft in range(KT):
            nc.vector.scalar_tensor_tensor(
                x232[:, ft, :], ypsl(ft), w["b2"][:, ft:ft + 1],
                x232[:, ft, :], op0=ALU.add, op1=ALU.add)
    else:
        yf = cx.act1.tile([128, KT, TOK], F32, name="yfld", tag="yfld")
        nc.sync.dma_start(yf[:], yf_sc[t])
        for ft in range(KT):
            yb = cx.tmp.tile([128, TOK], F32, name="yb", tag="yb")
            nc.vector.scalar_tensor_tensor(
                yb[:], ypsl(ft), w["b2"][:, ft:ft + 1], x232[:, ft, :],
                op0=ALU.add, op1=ALU.add)
            nc.vector.tensor_add(x232[:, ft, :], yb[:], yf[:, ft, :])
    nc.sync.dma_start(out_dst[t], x232[:])


# ---------------------------------------------------------------- entry point

@functools.cache
def _compiled_nc():
    return build_nc()


def kernel(**inputs):
    inputs = {k: np.asarray(v) for k, v in inputs.items()}
    nc = _compiled_nc()
    in_maps = make_in_maps(inputs)
    res = run_bass_kernel_spmd(nc, in_maps, list(range(NCORES)))
    return unshard_output(res.results)
